# revision 28
# baseline (speedup 1.0000x reference)
"""Trainium2 Bass kernel for a small autoregressive transformer block with
local-windowed causal attention and a large (16k) vocab head.

Data-parallel over batch: batch item b runs on NeuronCore b (8 cores).
Per core:
  h   = embed_tab[x] + pos                      [1024, 512]
  q/k/v = h @ Wq/k/v (+b)                       [1024, 512]
  s   = q @ k^T / sqrt(D) + local_causal_mask   (banded, window <= 298)
  o   = softmax(s) @ v @ Wo (+bo)
  h1  = LN(h + o);  f = relu(h1@W1+b1)@W2+b2;  h2 = LN(h1 + f)
  out = h2 @ Wh (+bh)                           [1024, 16384]

Body matmuls run as float32r (full-rate fp32 with N=512 moving dim). The
vocab head runs as fp8e4m3 DoubleRow (256-deep contraction at 0.5 cyc/row)
with a 3-pass residual split (a@w + a@dw + da@w, Wh pre-scaled by 64) to
stay within the 2e-2 error budget.
kernel(**inputs) takes full unsharded inputs, returns [8, 1024, 16384] f32.
"""

import math
import numpy as np

import concourse.bass as bass
import concourse.mybir as mybir
import concourse.tile as tile
from concourse import bacc
from concourse.bass_utils import run_bass_kernel_spmd
from concourse.masks import make_identity

# ---- problem constants (hardcoded per contract) ----
GH = 32
GW = 32
SEQ = 1024
WIN = 9
D = 512
DFF = 1024
VOCAB = 16384
EPS = 1e-5
NEG = -1e30

P = 128
NT = SEQ // P        # 8 token chunks
DC = D // P          # 4 d chunks
FC = DFF // P        # 8 dff chunks
NV = VOCAB // 512    # 32 vocab chunks
INV_SQRT_D = 1.0 / math.sqrt(D)

F32 = mybir.dt.float32
F32R = mybir.dt.float32r
BF16 = mybir.dt.bfloat16
FP8 = mybir.dt.float8e4
I32 = mybir.dt.int32
OUT_BF16 = True
WH_SCALE = 64.0  # Wh pre-scaled by 64 on host; folded out at PSUM eviction
DR = mybir.MatmulPerfMode.DoubleRow
AF = mybir.ActivationFunctionType


def _window_start(i: int) -> int:
    # k-window [ws, ws+512) covers all allowed keys for query chunk i
    # (max lookback is WIN*GW + WIN = 297 < 384).
    return 128 * max(0, i - 3)


def _mask_tiles() -> np.ndarray:
    idx = np.arange(SEQ)
    r, c = idx // GW, idx % GW
    allow = (
        (np.abs(r[:, None] - r[None, :]) <= WIN)
        & (np.abs(c[:, None] - c[None, :]) <= WIN)
        & (idx[None, :] <= idx[:, None])
    )
    maskf = np.where(allow, 0.0, NEG).astype(np.float32)
    tiles = np.empty((NT, P, 512), np.float32)
    for i in range(NT):
        ws = _window_start(i)
        tiles[i] = maskf[i * P : (i + 1) * P, ws : ws + 512]
    return tiles


def _r(ap):
    """bitcast to float32r for full-rate fp32 matmul."""
    return ap.bitcast(F32R)


def _bcast_ap(a: bass.AP) -> bass.AP:
    """[n] DRAM vector AP -> [P, n] partition-broadcast DMA source."""
    return bass.AP(tensor=a.tensor, offset=a.offset, ap=[[0, P], *a.ap])


def _build_program(flags: dict, wh_bufs: int = 6, msk_bufs: int = 6, lean: bool = False) -> bass.Bass:
    nc = bacc.Bacc("TRN2", target_bir_lowering=False)

    # ---------- I/O ----------
    x_d = nc.declare_dram_parameter("x", [SEQ], I32, False)
    emb_d = nc.declare_dram_parameter("emb", [VOCAB, D], BF16, False)
    pos_d = nc.declare_dram_parameter("pos", [SEQ, D], BF16, False)
    msk_d = nc.declare_dram_parameter("maskt", [NT, P, 512], BF16, False)
    wq_d = nc.declare_dram_parameter("wq", [D, D], F32, False)
    wk_d = nc.declare_dram_parameter("wk", [D, D], F32, False)
    wv_d = nc.declare_dram_parameter("wv", [D, D], F32, False)
    wo_d = nc.declare_dram_parameter("wo", [D, D], F32, False)
    w1_d = nc.declare_dram_parameter("w1", [D, DFF], F32, False)
    w2_d = nc.declare_dram_parameter("w2", [DFF, D], F32, False)
    # fp8 head weights: [p, vc, vhalf, ki2, i, 256] with d = 128*(2*ki2+i)+p
    wh8_d = nc.declare_dram_parameter("wh8", [P, NV, 2, 2, 2, 256], FP8, False)
    dwh8_d = nc.declare_dram_parameter("dwh8", [P, NV, 2, 2, 2, 256], FP8, False)
    dp = lambda name, shape: nc.declare_dram_parameter(name, shape, F32, False)
    bq_d = dp("bq", [D]) if flags["bq"] else None
    bk_d = dp("bk", [D]) if flags["bk"] else None
    bv_d = dp("bv", [D]) if flags["bv"] else None
    bo_d = dp("bo", [D]) if flags["bo"] else None
    b1_d = dp("b1", [DFF]) if flags["b1"] else None
    b2_d = dp("b2", [D]) if flags["b2"] else None
    bh_d = dp("bh", [VOCAB]) if flags["bh"] else None
    g1_d = dp("g1", [D]) if flags["g1"] else None
    be1_d = dp("be1", [D]) if flags["be1"] else None
    g2_d = dp("g2", [D]) if flags["g2"] else None
    be2_d = dp("be2", [D]) if flags["be2"] else None
    out_d = nc.declare_dram_parameter("out", [SEQ, VOCAB], BF16 if OUT_BF16 else F32, True)

    with tile.TileContext(nc) as tc:
        # ----- whole-kernel pools -----
        const = tc.alloc_tile_pool(name="const", bufs=1)
        small = tc.alloc_tile_pool(name="small", bufs=8)
        psum = tc.alloc_tile_pool(name="psA", bufs=6, space="PSUM")
        psum_t = tc.alloc_tile_pool(name="psT", bufs=2, space="PSUM")
        opool = tc.alloc_tile_pool(name="outev", bufs=2, side="right")
        p_h2T = tc.alloc_tile_pool(name="h2Tp", bufs=1, side="right")

        ident_f = const.tile([P, P], F32, tag="ident_f")
        ident = const.tile([P, P], F32R, tag="ident")
        eps_t = const.tile([P, 1], F32, tag="eps")
        nc.vector.memset(eps_t[:], EPS)
        x_sb = const.tile([P, NT], I32, tag="x_sb")
        nc.sync.dma_start(out=x_sb[:], in_=x_d[:].rearrange("(j p) -> p j", p=P))

        def load_col_bias(handle, nchunks, tag):
            # [nchunks*P] DRAM -> [P, nchunks] (chunk m in column m)
            t = const.tile([P, nchunks], F32, tag=tag)
            nc.sync.dma_start(out=t[:], in_=handle[:].rearrange("(m p) -> p m", p=P))
            return t

        def load_bcast(handle, n, tag):
            t = const.tile([P, n], F32, tag=tag)
            nc.sync.dma_start(out=t[:], in_=_bcast_ap(handle[:]))
            return t

        bq_sb = load_col_bias(bq_d, DC, "bq") if bq_d else None
        bk_sb = load_col_bias(bk_d, DC, "bk") if bk_d else None
        b1_sb = load_col_bias(b1_d, FC, "b1") if b1_d else None
        bv_bc = load_bcast(bv_d, D, "bv") if bv_d else None
        bo_bc = load_bcast(bo_d, D, "bo") if bo_d else None
        b2_bc = load_bcast(b2_d, D, "b2") if b2_d else None
        g1_bc = load_bcast(g1_d, D, "g1") if g1_d else None
        be1_bc = load_bcast(be1_d, D, "be1") if be1_d else None
        g2_bc = load_bcast(g2_d, D, "g2") if g2_d else None
        be2_bc = load_bcast(be2_d, D, "be2") if be2_d else None

        a8T = [p_h2T.tile([P, DC, P], FP8, tag=f"a8T{j}", name=f"a8T{j}") for j in range(NT)]
        da8T = [p_h2T.tile([P, DC, P], FP8, tag=f"da8T{j}", name=f"da8T{j}") for j in range(NT)]

        # ----- phase A pools (left, LIFO) -----
        p_woh = tc.alloc_tile_pool(name="woh", bufs=1)         # wo, h  (-> stage 4)
        wo_sb = p_woh.tile([P, DC, D], F32R, tag="wo")
        h_sb = p_woh.tile([P, NT, D], F32R, tag="h")

        p_oT = tc.alloc_tile_pool(name="oTp", bufs=1)          # oT    (-> stage 4)
        oT = p_oT.tile([P, DC, SEQ], F32R, tag="oT")

        p_v = tc.alloc_tile_pool(name="vp", bufs=1)            # v (-> wave 2)
        v_sb = p_v.tile([P, NT, D], F32R, tag="v")
        p_at = tc.alloc_tile_pool(name="attnw", bufs=3)        # softmax work (-> stage 4)
        p_qk = tc.alloc_tile_pool(name="qkp", bufs=1)          # qT,kT (-> wave 1)
        qT = p_qk.tile([P, DC, SEQ], F32R, tag="qT")
        kT = p_qk.tile([P, DC, SEQ], F32R, tag="kT")

        p_wq = tc.alloc_tile_pool(name="wqp", bufs=1)          # wq,wk,wv,hT (-> stage 2)
        wq_sb = p_wq.tile([P, DC, D], F32R, tag="wq")
        wk_sb = p_wq.tile([P, DC, D], F32R, tag="wk")
        wv_sb = p_wq.tile([P, DC, D], F32R, tag="wv")
        hT = p_wq.tile([P, DC, SEQ], F32R, tag="hT")

        # ---------- stage 1: embedding gather + positional + transpose ----------
        # interleave gather_j / pos_j DMA issue so chunk j's inputs land together
        hb_ts = []
        pos_ts = []
        for jj in range(NT):
            hb_t = p_wq.tile([P, D], BF16, tag="hb", bufs=NT, name=f"hb{jj}")
            nc.gpsimd.indirect_dma_start(
                out=hb_t[:],
                out_offset=None,
                in_=emb_d[:],
                in_offset=bass.IndirectOffsetOnAxis(ap=x_sb[:, jj : jj + 1], axis=0),
            )
            hb_ts.append(hb_t)
            pos_t = p_wq.tile([P, D], BF16, tag="pos", bufs=NT, name=f"pos{jj}")
            nc.sync.dma_start(out=pos_t[:], in_=pos_d[jj * P : (jj + 1) * P, :])
            pos_ts.append(pos_t)

        make_identity(nc, ident_f[:])
        nc.vector.tensor_copy(out=ident[:], in_=ident_f[:])

        def s1_add(j):
            nc.vector.tensor_add(out=h_sb[:, j, :], in0=hb_ts[j][:], in1=pos_ts[j][:])

        def s1_trans(j):
            pt = psum_t.tile([P, 512], F32, tag="pt", name=f"s1pt{j}")
            for m in range(DC):
                nc.tensor.transpose(
                    out=_r(pt[:, m * P : (m + 1) * P]),
                    in_=_r(h_sb[:, j, m * P : (m + 1) * P]),
                    identity=_r(ident[:]),
                )
            nc.scalar.copy(out=hT[:, :, j * P : (j + 1) * P], in_=pt[:])

        for k in range(NT + 1):
            if k < NT:
                s1_add(k)
            if k >= 1:
                s1_trans(k - 1)

        # weight DMAs issued after stage-1 loads so embeddings/pos win the queue
        for w_sb, w_d in ((wq_sb, wq_d), (wk_sb, wk_d), (wv_sb, wv_d), (wo_sb, wo_d)):
            nc.sync.dma_start(out=w_sb[:], in_=_r(w_d[:].rearrange("(k p) o -> p k o", p=P)))

        # ---------- stage 2: qT / kT (d-major), v (token-major) ----------
        # t-major order: all groups needing hT[0:512] first (PE is in-order)
        for t in range(SEQ // 512):
            for (wt, bt, dst) in ((wq_sb, bq_sb, qT), (wk_sb, bk_sb, kT)):
                for m in range(DC):
                    ps = psum.tile([P, 512], F32, tag="ps")
                    for ki in range(DC):
                        nc.tensor.matmul(
                            ps[:],
                            _r(wt[:, ki, m * P : (m + 1) * P]),
                            _r(hT[:, ki, t * 512 : (t + 1) * 512]),
                            start=(ki == 0),
                            stop=(ki == DC - 1),
                        )
                    dslc = dst[:, m, t * 512 : (t + 1) * 512]
                    if bt is not None:
                        nc.scalar.activation(
                            out=dslc, in_=ps[:], func=AF.Identity,
                            bias=bt[:, m : m + 1], scale=1.0,
                        )
                    elif dst is kT:
                        nc.vector.tensor_copy(out=dslc, in_=ps[:])
                    else:
                        nc.scalar.copy(out=dslc, in_=ps[:])
            for j in range(4 * t, 4 * t + 4):
                ps = psum.tile([P, 512], F32, tag="ps")
                for ki in range(DC):
                    nc.tensor.matmul(
                        ps[:],
                        _r(hT[:, ki, j * P : (j + 1) * P]),
                        _r(wv_sb[:, ki, :]),
                        start=(ki == 0),
                        stop=(ki == DC - 1),
                    )
                if bv_bc is not None:
                    nc.vector.tensor_add(out=v_sb[:, j, :], in0=ps[:], in1=bv_bc[:])
                else:
                    nc.vector.tensor_copy(out=v_sb[:, j, :], in_=ps[:])

        p_wq.release()

        # ---------- stage 3 wave 1: scores + softmax ----------
        attns = []
        recips = []
        for i in range(NT):
            ws = _window_start(i)
            nw = min(512, max(256, (i + 1) * P))  # live window (>=256 keeps f32r fast)
            ps_s = psum.tile([P, 512], F32, tag="ps")
            for ki in range(DC):
                nc.tensor.matmul(
                    ps_s[:, :nw],
                    _r(qT[:, ki, i * P : (i + 1) * P]),
                    _r(kT[:, ki, ws : ws + nw]),
                    start=(ki == 0),
                    stop=(ki == DC - 1),
                )
            msk_t = p_at.tile([P, 512], BF16, tag="msk", bufs=4)
            nc.sync.dma_start(out=msk_t[:], in_=msk_d[i])
            s_t = p_at.tile([P, 512], F32, tag="s_t", bufs=2)
            nc.vector.tensor_add(out=s_t[:, :nw], in0=ps_s[:, :nw], in1=msk_t[:, :nw])
            attn = p_at.tile([P, 512], F32R, tag="attn", bufs=NT, name=f"attn{i}")
            denom = small.tile([P, 1], F32, tag="denom")
            nc.scalar.activation(
                out=attn[:, :nw], in_=s_t[:, :nw], func=AF.Exp,
                bias=0.0, scale=INV_SQRT_D,
                accum_out=denom[:, 0:1],
            )
            recip = small.tile([P, 1], F32, tag="recip", bufs=NT, name=f"recip{i}")
            nc.vector.reciprocal(out=recip[:], in_=denom[:])
            attns.append(attn)
            recips.append(recip)

        p_qk.release()

        # ----- right-side pools for FFN phase -----
        whpool = tc.alloc_tile_pool(name="whstream", bufs=8, side="right")
        p_h1 = tc.alloc_tile_pool(name="h1p", bufs=1, side="right")
        h1_sb = p_h1.tile([P, NT, D], F32R, tag="h1")
        h1T = p_h1.tile([P, DC, SEQ], F32R, tag="h1T")
        w1_sb = p_h1.tile([P, DC, DFF], F32R, tag="w1")
        nc.sync.dma_start(out=w1_sb[:], in_=_r(w1_d[:].rearrange("(k p) o -> p k o", p=P)))

        # ---------- stage 3 wave 2 + stage 4, software-pipelined ----------
        p_st4 = tc.alloc_tile_pool(name="st4", bufs=3)
        attnTs = [None] * NT
        o_ts = [None] * NT

        def w2_a(i):  # attn transposes + attnT eviction
            ws = _window_start(i)
            kb0 = ws // P
            nkb = min(DC, i - kb0 + 1)
            pt = psum_t.tile([P, 512], F32, tag="pt", name=f"atp{i}")
            for kk in range(nkb):
                nc.tensor.transpose(
                    out=_r(pt[:, kk * P : (kk + 1) * P]),
                    in_=_r(attns[i][:, kk * P : (kk + 1) * P]),
                    identity=_r(ident[:]),
                )
            attnT = p_at.tile([P, 512], F32R, tag="attnT", bufs=3, name=f"attnT{i}")
            nc.vector.tensor_copy(out=attnT[:, : nkb * P], in_=pt[:, : nkb * P])
            attnTs[i] = attnT

        def w2_b(i):  # o matmuls + scale
            ws = _window_start(i)
            kb0 = ws // P
            nkb = min(DC, i - kb0 + 1)
            ps_o = psum.tile([P, 512], F32, tag="ps", name=f"pso{i}")
            for kk in range(nkb):
                nc.tensor.matmul(
                    ps_o[:],
                    attnTs[i][:, kk * P : (kk + 1) * P],
                    _r(v_sb[:, kb0 + kk, :]),
                    start=(kk == 0),
                    stop=(kk == nkb - 1),
                )
            o_t = p_at.tile([P, D], F32R, tag="o_t", bufs=3, name=f"o_t{i}")
            nc.vector.tensor_scalar_mul(out=o_t[:], in0=ps_o[:], scalar1=recips[i][:, 0:1])
            o_ts[i] = o_t

        def w2_c(i):  # oT transposes + eviction
            pt2 = psum_t.tile([P, 512], F32, tag="pt", name=f"otp{i}")
            for m in range(DC):
                nc.tensor.transpose(
                    out=_r(pt2[:, m * P : (m + 1) * P]),
                    in_=_r(o_ts[i][:, m * P : (m + 1) * P]),
                    identity=_r(ident[:]),
                )
            nc.vector.tensor_copy(out=oT[:, :, i * P : (i + 1) * P], in_=pt2[:])

        def s4_proj(j):  # attn projection + residual + LN1 (no transpose)
            ps = psum.tile([P, 512], F32, tag="ps", name=f"psp{j}")
            for m in range(DC):
                nc.tensor.matmul(
                    ps[:],
                    _r(oT[:, m, j * P : (j + 1) * P]),
                    _r(wo_sb[:, m, :]),
                    start=(m == 0),
                    stop=(m == DC - 1),
                )
            r1 = p_st4.tile([P, D], F32, tag="r1", name=f"r1_{j}")
            nc.vector.tensor_add(out=r1[:], in0=h_sb[:, j, :], in1=ps[:])
            if bo_bc is not None:
                nc.vector.tensor_add(out=r1[:], in0=r1[:], in1=bo_bc[:])
            stats = small.tile([P, 6], F32, tag="stats")
            nc.vector.bn_stats(out=stats[:], in_=r1[:])
            mv = small.tile([P, 2], F32, tag="mv")
            nc.vector.bn_aggr(out=mv[:], in_=stats[:])
            stdt = small.tile([P, 1], F32, tag="stdt")
            nc.scalar.activation(
                out=stdt[:], in_=mv[:, 1:2], func=AF.Sqrt,
                bias=eps_t[:, 0:1], scale=1.0,
            )
            rstd = small.tile([P, 1], F32, tag="rstd")
            nc.vector.reciprocal(out=rstd[:], in_=stdt[:])
            nc.vector.tensor_scalar(
                out=h1_sb[:, j, :], in0=r1[:],
                scalar1=mv[:, 0:1], scalar2=rstd[:, 0:1],
                op0=mybir.AluOpType.subtract, op1=mybir.AluOpType.mult,
            )
            if g1_bc is not None:
                nc.vector.tensor_mul(out=h1_sb[:, j, :], in0=h1_sb[:, j, :], in1=g1_bc[:])
            if be1_bc is not None:
                nc.vector.tensor_add(out=h1_sb[:, j, :], in0=h1_sb[:, j, :], in1=be1_bc[:])

        def s4_trans(j):  # h1 transposes + h1T eviction
            pt3 = psum_t.tile([P, 512], F32, tag="pt", name=f"h1p{j}")
            for m in range(DC):
                nc.tensor.transpose(
                    out=_r(pt3[:, m * P : (m + 1) * P]),
                    in_=_r(h1_sb[:, j, m * P : (m + 1) * P]),
                    identity=_r(ident[:]),
                )
            nc.scalar.copy(out=h1T[:, :, j * P : (j + 1) * P], in_=pt3[:])

        for k in range(NT + 4):
            if k < NT:
                w2_a(k)
            if 1 <= k < NT + 1:
                w2_b(k - 1)
            if 2 <= k < NT + 2:
                w2_c(k - 2)
            if 3 <= k < NT + 3:
                s4_proj(k - 3)
            if 4 <= k:
                s4_trans(k - 4)

        p_st4.release()
        p_at.release()
        p_v.release()
        p_oT.release()
        p_woh.release()

        p_w12 = tc.alloc_tile_pool(name="w12", bufs=1, side="right")
        w2_sb = p_w12.tile([P, FC, D], F32R, tag="w2")
        nc.sync.dma_start(out=w2_sb[:], in_=_r(w2_d[:].rearrange("(k p) o -> p k o", p=P)))

        # ---------- stage 5: FFN up, f1T = relu(W1^T @ h1T + b1) ----------
        p_f1 = tc.alloc_tile_pool(name="f1p", bufs=1, side="right")
        f1T = p_f1.tile([P, FC, SEQ], F32R, tag="f1T")
        def ffn1_group(n, t):
            ps = psum.tile([P, 512], F32, tag="ps", name=f"psf{n}_{t}")
            for ki in range(DC):
                nc.tensor.matmul(
                    ps[:],
                    _r(w1_sb[:, ki, n * P : (n + 1) * P]),
                    _r(h1T[:, ki, t * 512 : (t + 1) * 512]),
                    start=(ki == 0),
                    stop=(ki == DC - 1),
                )
            fslc = f1T[:, n, t * 512 : (t + 1) * 512]
            if b1_sb is not None:
                nc.vector.tensor_scalar(
                    out=fslc, in0=ps[:],
                    scalar1=b1_sb[:, n : n + 1], scalar2=0.0,
                    op0=mybir.AluOpType.add, op1=mybir.AluOpType.max,
                )
            else:
                nc.vector.tensor_scalar_max(out=fslc, in0=ps[:], scalar1=0.0)

        # ---------- stage 6: FFN down + residual + LN2 (pipelined) ----------
        def s6_main(j):
            ps = psum.tile([P, 512], F32, tag="ps", name=f"ps6_{j}")
            for n in range(FC):
                nc.tensor.matmul(
                    ps[:],
                    _r(f1T[:, n, j * P : (j + 1) * P]),
                    _r(w2_sb[:, n, :]),
                    start=(n == 0),
                    stop=(n == FC - 1),
                )
            r2 = p_f1.tile([P, D], F32, tag="r2", bufs=3, name=f"r2_{j}")
            nc.vector.tensor_add(out=r2[:], in0=h1_sb[:, j, :], in1=ps[:])
            if b2_bc is not None:
                nc.vector.tensor_add(out=r2[:], in0=r2[:], in1=b2_bc[:])
            stats = small.tile([P, 6], F32, tag="stats")
            nc.vector.bn_stats(out=stats[:], in_=r2[:])
            mv = small.tile([P, 2], F32, tag="mv")
            nc.vector.bn_aggr(out=mv[:], in_=stats[:])
            stdt = small.tile([P, 1], F32, tag="stdt")
            nc.scalar.activation(
                out=stdt[:], in_=mv[:, 1:2], func=AF.Sqrt,
                bias=eps_t[:, 0:1], scale=1.0,
            )
            rstd = small.tile([P, 1], F32, tag="rstd")
            nc.vector.reciprocal(out=rstd[:], in_=stdt[:])
            h2_t = p_f1.tile([P, D], F32R, tag="h2_t", bufs=3, name=f"h2t_{j}")
            nc.vector.tensor_scalar(
                out=h2_t[:], in0=r2[:],
                scalar1=mv[:, 0:1], scalar2=rstd[:, 0:1],
                op0=mybir.AluOpType.subtract, op1=mybir.AluOpType.mult,
            )
            if g2_bc is not None:
                nc.vector.tensor_mul(out=h2_t[:], in0=h2_t[:], in1=g2_bc[:])
            if be2_bc is not None:
                nc.vector.tensor_add(out=h2_t[:], in0=h2_t[:], in1=be2_bc[:])
            return h2_t

        h2ts = [None] * NT

        def s6_trans(j):
            # transpose h2, then split-quantize to fp8: a8T = fp8(h2T),
            # da8T = fp8(h2T - a8T)
            pt = psum_t.tile([P, DC, P], F32, tag="pt", name=f"h2p{j}")
            for m in range(DC):
                nc.tensor.transpose(
                    out=_r(pt[:, m, :]),
                    in_=_r(h2ts[j][:, m * P : (m + 1) * P]),
                    identity=_r(ident[:]),
                )
            nc.scalar.copy(out=a8T[j][:, :, :], in_=pt[:, :, :])
            da_t = p_f1.tile([P, DC, P], F32, tag="da_t", bufs=2, name=f"da_t{j}")
            nc.vector.tensor_sub(out=da_t[:, :, :], in0=pt[:, :, :], in1=a8T[j][:, :, :])
            nc.gpsimd.tensor_copy(out=da8T[j][:, :, :], in_=da_t[:, :, :])

        # head chunks for vc=0,1 interleaved into stage-6 so PE fills LN waits
        def load_whv(vc, nm):
            whv = whpool.tile([P, 2, 2, 2, 256], FP8, tag="whv", name=f"whv{nm}")
            nc.sync.dma_start(out=whv[:], in_=wh8_d[:, vc])
            dwv = whpool.tile([P, 2, 2, 2, 256], FP8, tag="dwv", name=f"dwv{nm}")
            nc.sync.dma_start(out=dwv[:], in_=dwh8_d[:, vc])
            return whv, dwv

        whv0, dwv0 = load_whv(0, "0")
        otile0 = opool.tile([P, NT, 512], BF16 if OUT_BF16 else F32, tag="ot", name="otile0")
        whv1, dwv1 = load_whv(1, "1")
        otile1 = opool.tile([P, NT, 512], BF16 if OUT_BF16 else F32, tag="ot", name="otile1")

        def head_j(whv, dwv, otile, j, toggle):
            # 3-pass fp8 DoubleRow: a@w + a@dw + da@w, one PSUM group per
            # 256-vocab half; scale 1/WH_SCALE folded into the eviction
            ps = psum.tile([P, 512], F32, tag="ps", name=f"psh{toggle}_{j}")
            for t in range(2):
                ops = []
                for ki2 in range(2):
                    lhs_a = a8T[j][:, 2 * ki2 : 2 * ki2 + 2, :]
                    lhs_da = da8T[j][:, 2 * ki2 : 2 * ki2 + 2, :]
                    ops.append((lhs_a, whv[:, t, ki2]))
                    ops.append((lhs_a, dwv[:, t, ki2]))
                    ops.append((lhs_da, whv[:, t, ki2]))
                for n, (l, r) in enumerate(ops):
                    nc.tensor.matmul(
                        ps[:, t * 256 : (t + 1) * 256],
                        l,
                        r,
                        start=(n == 0),
                        stop=(n == len(ops) - 1),
                        perf_mode=DR,
                    )
            if bh_sb_for(toggle) is not None:
                nc.vector.tensor_scalar_mul(
                    out=otile[:, j, :], in0=ps[:], scalar1=1.0 / WH_SCALE
                )
                nc.vector.tensor_add(
                    out=otile[:, j, :], in0=otile[:, j, :], in1=bh_sb_for(toggle)[:]
                )
            elif j % 2 == 0:
                nc.vector.tensor_scalar_mul(
                    out=otile[:, j, :], in0=ps[:], scalar1=1.0 / WH_SCALE
                )
            else:
                nc.scalar.activation(
                    out=otile[:, j, :], in_=ps[:], func=AF.Identity,
                    bias=0.0, scale=1.0 / WH_SCALE,
                )

        _bh_tiles = {}

        def bh_sb_for(key):
            return _bh_tiles.get(key)

        if bh_d is not None:
            bh0 = whpool.tile([P, 512], F32, tag="bh", bufs=2, name="bh0")
            nc.sync.dma_start(out=bh0[:], in_=_bcast_ap(bh_d[0:512]))
            _bh_tiles[0] = bh0
            bh1 = whpool.tile([P, 512], F32, tag="bh", bufs=2, name="bh1")
            nc.sync.dma_start(out=bh1[:], in_=_bcast_ap(bh_d[512:1024]))
            _bh_tiles[1] = bh1

        for t in range(SEQ // 512):
            for n in range(FC):
                ffn1_group(n, t)
                if t == 1 and n % 2 == 1:
                    j = n // 2
                    h2ts[j] = s6_main(j)

        for k in range(NT + 3):
            if 4 <= k < NT:
                h2ts[k] = s6_main(k)
            if 1 <= k <= NT:
                s6_trans(k - 1)
            if 2 <= k <= NT + 1:
                head_j(whv0, dwv0, otile0, k - 2, 0)
            if 3 <= k <= NT + 2:
                head_j(whv1, dwv1, otile1, k - 3, 1)
        out_rr = out_d[:].rearrange("(j p) v -> p j v", p=P)
        nc.sync.dma_start(out=out_rr[:, :, 0:512], in_=otile0[:])
        nc.sync.dma_start(out=out_rr[:, :, 512:1024], in_=otile1[:])

        p_f1.release()
        p_w12.release()
        p_h1.release()

        # ---------- stage 7: vocab head (vc >= 2) ----------
        out_r = out_d[:].rearrange("(j p) v -> p j v", p=P)
        for vc in range(2, NV):
            whv, dwv = load_whv(vc, str(vc))
            if bh_d is not None:
                bh_bc = whpool.tile([P, 512], F32, tag="bh", bufs=2, name=f"bh{vc}")
                nc.sync.dma_start(
                    out=bh_bc[:], in_=_bcast_ap(bh_d[vc * 512 : (vc + 1) * 512])
                )
                _bh_tiles[vc] = bh_bc
            otile = opool.tile([P, NT, 512], BF16 if OUT_BF16 else F32, tag="ot")
            vs = slice(vc * 512, (vc + 1) * 512)
            # split stores per j-half (last chunk: per j-pair) to shrink the
            # final DMA drain after the last matmul
            if vc == NV - 1:
                for j in range(NT):
                    head_j(whv, dwv, otile, j, vc)
                    if j % 2 == 1:
                        nc.sync.dma_start(
                            out=out_r[:, j - 1 : j + 1, vs],
                            in_=otile[:, j - 1 : j + 1, :],
                        )
            else:
                for j in range(NT):
                    head_j(whv, dwv, otile, j, vc)
                    if j == NT // 2 - 1 or j == NT - 1:
                        h0 = j + 1 - NT // 2
                        nc.sync.dma_start(
                            out=out_r[:, h0 : j + 1, vs],
                            in_=otile[:, h0 : j + 1, :],
                        )

        whpool.release()
        p_h2T.release()
        opool.release()
        psum_t.release()
        psum.release()
        small.release()
        const.release()

    nc.finalize()
    return nc


_PROGRAM_CACHE: dict = {}


def _get_program(flags: dict) -> bass.Bass:
    key = tuple(sorted(flags.items()))
    if key not in _PROGRAM_CACHE:
        _PROGRAM_CACHE[key] = _build_program(flags)
    return _PROGRAM_CACHE[key]


def _prep(x, embed_tab, row_embed, col_embed, Wq, bq, Wk, bk, Wv, bv, Wo, bo,
          ln1_g, ln1_b, W1, b1, W2, b2, ln2_g, ln2_b, Wh, bh):
    """Shared host-side prep: flags, common input map, per-core x shards."""
    f32c = lambda a: np.ascontiguousarray(np.asarray(a, dtype=np.float32))
    x = np.asarray(x)
    B = x.shape[0]
    assert x.shape == (B, SEQ)

    import ml_dtypes

    bf16 = ml_dtypes.bfloat16
    fp8 = ml_dtypes.float8_e4m3
    arrs = dict(
        wq=f32c(Wq), wk=f32c(Wk), wv=f32c(Wv), wo=f32c(Wo),
        w1=f32c(W1), w2=f32c(W2),
    )
    arrs["emb"] = np.ascontiguousarray(f32c(embed_tab).astype(bf16))
    # fp8 split head weights, pre-scaled by WH_SCALE:
    #   wh8 = fp8(Wh*S), dwh8 = fp8(Wh*S - wh8); layout [p, vc, t, ki2, i, n]
    whs = f32c(Wh) * WH_SCALE
    w8 = whs.astype(fp8)
    dw8 = (whs - w8.astype(np.float32)).astype(fp8)

    def _wh_layout(a):
        # [D=512, V] -> [ki2, i, p, vc, t, n] -> [p, vc, t, ki2, i, n]
        a = a.reshape(2, 2, P, NV, 2, 256)
        return np.ascontiguousarray(a.transpose(2, 3, 4, 0, 1, 5))

    arrs["wh8"] = _wh_layout(w8)
    arrs["dwh8"] = _wh_layout(dw8)
    pos = np.concatenate(
        [np.repeat(f32c(row_embed), GW, axis=0), np.tile(f32c(col_embed), (GH, 1))],
        axis=-1,
    )
    arrs["pos"] = np.ascontiguousarray(pos.astype(bf16))
    arrs["maskt"] = np.ascontiguousarray(_mask_tiles().astype(bf16))

    bias_map = dict(
        bq=f32c(bq), bk=f32c(bk), bv=f32c(bv), bo=f32c(bo), b1=f32c(b1),
        b2=f32c(b2), bh=f32c(bh), be1=f32c(ln1_b), be2=f32c(ln2_b),
    )
    gain_map = dict(g1=f32c(ln1_g), g2=f32c(ln2_g))
    flags = {k: bool(np.any(v)) for k, v in bias_map.items()}
    flags.update({k: bool(np.any(v != 1.0)) for k, v in gain_map.items()})
    for k, v in {**bias_map, **gain_map}.items():
        if flags[k]:
            arrs[k] = v

    xs = [np.ascontiguousarray(x[c].astype(np.int32)) for c in range(B)]
    return flags, arrs, xs, B


def kernel(**inputs):
    flags, arrs, xs, B = _prep(**inputs)
    nc = _get_program(flags)
    core_ids = list(range(8))
    in_maps = [{**arrs, "x": xs[c % B]} for c in core_ids]
    res = run_bass_kernel_spmd(nc, in_maps, core_ids)
    out = np.stack([res.results[c]["out"] for c in range(B)], axis=0)
    return np.asarray(out, dtype=np.float32)



# revision 44
# speedup vs baseline: 1.0275x; 1.0275x over previous
"""Trainium2 Bass kernel for a small autoregressive transformer block with
local-windowed causal attention and a large (16k) vocab head.

Data-parallel over batch: batch item b runs on NeuronCore b (8 cores).
Per core:
  h   = embed_tab[x] + pos                      [1024, 512]
  q/k/v = h @ Wq/k/v (+b)                       [1024, 512]
  s   = q @ k^T / sqrt(D) + local_causal_mask   (banded, window <= 298)
  o   = softmax(s) @ v @ Wo (+bo)
  h1  = LN(h + o);  f = relu(h1@W1+b1)@W2+b2;  h2 = LN(h1 + f)
  out = h2 @ Wh (+bh)                           [1024, 16384]

Body matmuls run as float32r (full-rate fp32 with N=512 moving dim). The
vocab head runs as fp8e4m3 DoubleRow (256-deep contraction at 0.5 cyc/row)
with a 3-pass residual split (a@w + a@dw + da@w, Wh pre-scaled by 64) to
stay within the 2e-2 error budget.
kernel(**inputs) takes full unsharded inputs, returns [8, 1024, 16384] f32.
"""

import math
import numpy as np

import concourse.bass as bass
import concourse.mybir as mybir
import concourse.tile as tile
from concourse import bacc
from concourse.bass_utils import run_bass_kernel_spmd
from concourse.masks import make_identity

# ---- problem constants (hardcoded per contract) ----
GH = 32
GW = 32
SEQ = 1024
WIN = 9
D = 512
DFF = 1024
VOCAB = 16384
EPS = 1e-5
NEG = -1e30

P = 128
NT = SEQ // P        # 8 token chunks
DC = D // P          # 4 d chunks
FC = DFF // P        # 8 dff chunks
NV = VOCAB // 512    # 32 vocab chunks
INV_SQRT_D = 1.0 / math.sqrt(D)

F32 = mybir.dt.float32
F32R = mybir.dt.float32r
BF16 = mybir.dt.bfloat16
FP8 = mybir.dt.float8e4
I32 = mybir.dt.int32
OUT_BF16 = True
WH_SCALE = 64.0  # Wh pre-scaled by 64 on host; folded out at PSUM eviction
DR = mybir.MatmulPerfMode.DoubleRow
AF = mybir.ActivationFunctionType


def _window_start(i: int) -> int:
    # k-window [ws, ws+512) covers all allowed keys for query chunk i
    # (max lookback is WIN*GW + WIN = 297 < 384).
    return 128 * max(0, i - 3)


def _mask_tiles() -> np.ndarray:
    idx = np.arange(SEQ)
    r, c = idx // GW, idx % GW
    allow = (
        (np.abs(r[:, None] - r[None, :]) <= WIN)
        & (np.abs(c[:, None] - c[None, :]) <= WIN)
        & (idx[None, :] <= idx[:, None])
    )
    maskf = np.where(allow, 0.0, NEG).astype(np.float32)
    tiles = np.empty((NT, P, 512), np.float32)
    for i in range(NT):
        ws = _window_start(i)
        tiles[i] = maskf[i * P : (i + 1) * P, ws : ws + 512]
    return tiles


def _r(ap):
    """bitcast to float32r for full-rate fp32 matmul."""
    return ap.bitcast(F32R)


def _bcast_ap(a: bass.AP) -> bass.AP:
    """[n] DRAM vector AP -> [P, n] partition-broadcast DMA source."""
    return bass.AP(tensor=a.tensor, offset=a.offset, ap=[[0, P], *a.ap])


def _build_program(flags: dict, wh_bufs: int = 6, msk_bufs: int = 6, lean: bool = False) -> bass.Bass:
    nc = bacc.Bacc("TRN2", target_bir_lowering=False)

    # ---------- I/O ----------
    x_d = nc.declare_dram_parameter("x", [SEQ], I32, False)
    emb_d = nc.declare_dram_parameter("emb", [VOCAB, D], BF16, False)
    pos_d = nc.declare_dram_parameter("pos", [SEQ, D], BF16, False)
    msk_d = nc.declare_dram_parameter("maskt", [NT, P, 512], BF16, False)
    wq_d = nc.declare_dram_parameter("wq", [D, D], F32, False)
    wk_d = nc.declare_dram_parameter("wk", [D, D], F32, False)
    wv_d = nc.declare_dram_parameter("wv", [D, D], F32, False)
    wo_d = nc.declare_dram_parameter("wo", [D, D], F32, False)
    w1_d = nc.declare_dram_parameter("w1", [D, DFF], BF16, False)
    w2_d = nc.declare_dram_parameter("w2", [DFF, D], BF16, False)
    # fp8 head weights: [p, vc, vhalf, ki2, i, 256] with d = 128*(2*ki2+i)+p
    wh8_d = nc.declare_dram_parameter("wh8", [P, NV, 2, 2, 2, 256], FP8, False)
    dwh8_d = nc.declare_dram_parameter("dwh8", [P, NV, 2, 2, 2, 256], FP8, False)
    dp = lambda name, shape: nc.declare_dram_parameter(name, shape, F32, False)
    bq_d = dp("bq", [D]) if flags["bq"] else None
    bk_d = dp("bk", [D]) if flags["bk"] else None
    bv_d = dp("bv", [D]) if flags["bv"] else None
    bo_d = dp("bo", [D]) if flags["bo"] else None
    b1_d = dp("b1", [DFF]) if flags["b1"] else None
    b2_d = dp("b2", [D]) if flags["b2"] else None
    bh_d = dp("bh", [VOCAB]) if flags["bh"] else None
    g1_d = dp("g1", [D]) if flags["g1"] else None
    be1_d = dp("be1", [D]) if flags["be1"] else None
    g2_d = dp("g2", [D]) if flags["g2"] else None
    be2_d = dp("be2", [D]) if flags["be2"] else None
    out_d = nc.declare_dram_parameter("out", [SEQ, VOCAB], BF16 if OUT_BF16 else F32, True)

    with tile.TileContext(nc) as tc:
        # ----- whole-kernel pools -----
        const = tc.alloc_tile_pool(name="const", bufs=1)
        small = tc.alloc_tile_pool(name="small", bufs=8)
        psum = tc.alloc_tile_pool(name="psA", bufs=5, space="PSUM")
        psum_t = tc.alloc_tile_pool(name="psT", bufs=2, space="PSUM")
        opool = tc.alloc_tile_pool(name="outev", bufs=3, side="right")
        p_h2T = tc.alloc_tile_pool(name="h2Tp", bufs=1, side="right")

        ident_f = const.tile([P, P], F32, tag="ident_f")
        ident = const.tile([P, P], F32R, tag="ident")
        ident_b = const.tile([P, P], BF16, tag="ident_b")
        eps_t = const.tile([P, 1], F32, tag="eps")
        nc.vector.memset(eps_t[:], EPS)
        x_sb = const.tile([P, NT], I32, tag="x_sb")
        nc.sync.dma_start(out=x_sb[:], in_=x_d[:].rearrange("(j p) -> p j", p=P))

        def load_col_bias(handle, nchunks, tag):
            # [nchunks*P] DRAM -> [P, nchunks] (chunk m in column m)
            t = const.tile([P, nchunks], F32, tag=tag)
            nc.sync.dma_start(out=t[:], in_=handle[:].rearrange("(m p) -> p m", p=P))
            return t

        def load_bcast(handle, n, tag):
            t = const.tile([P, n], F32, tag=tag)
            nc.sync.dma_start(out=t[:], in_=_bcast_ap(handle[:]))
            return t

        bq_sb = load_col_bias(bq_d, DC, "bq") if bq_d else None
        bk_sb = load_col_bias(bk_d, DC, "bk") if bk_d else None
        b1_sb = load_col_bias(b1_d, FC, "b1") if b1_d else None
        bv_bc = load_bcast(bv_d, D, "bv") if bv_d else None
        bo_bc = load_bcast(bo_d, D, "bo") if bo_d else None
        b2_bc = load_bcast(b2_d, D, "b2") if b2_d else None
        g1_bc = load_bcast(g1_d, D, "g1") if g1_d else None
        be1_bc = load_bcast(be1_d, D, "be1") if be1_d else None
        g2_bc = load_bcast(g2_d, D, "g2") if g2_d else None
        be2_bc = load_bcast(be2_d, D, "be2") if be2_d else None

        a8T = [p_h2T.tile([P, DC, P], FP8, tag=f"a8T{j}", name=f"a8T{j}") for j in range(NT)]
        da8T = [p_h2T.tile([P, DC, P], FP8, tag=f"da8T{j}", name=f"da8T{j}") for j in range(NT)]

        # ----- phase A pools (left, LIFO) -----
        p_woh = tc.alloc_tile_pool(name="woh", bufs=1)         # wo, h  (-> stage 4)
        wo_sb = p_woh.tile([P, DC, D], F32R, tag="wo")
        h_sb = p_woh.tile([P, NT, D], F32R, tag="h")

        p_oT = tc.alloc_tile_pool(name="oTp", bufs=1)          # oT    (-> stage 4)
        oT = p_oT.tile([P, DC, SEQ], F32R, tag="oT")

        p_v = tc.alloc_tile_pool(name="vp", bufs=1)            # v (-> wave 2)
        v_sb = p_v.tile([P, NT, D], BF16, tag="v")
        p_at = tc.alloc_tile_pool(name="attnw", bufs=3)        # softmax work (-> stage 4)
        p_qk = tc.alloc_tile_pool(name="qkp", bufs=1)          # qT,kT (-> wave 1)
        qT = p_qk.tile([P, DC, SEQ], F32R, tag="qT")
        kT = p_qk.tile([P, DC, SEQ], F32R, tag="kT")

        p_wq = tc.alloc_tile_pool(name="wqp", bufs=1)          # wq,wk,wv,hT (-> stage 2)
        wq_sb = p_wq.tile([P, DC, D], F32R, tag="wq")
        wk_sb = p_wq.tile([P, DC, D], F32R, tag="wk")
        wv_sb = p_wq.tile([P, DC, D], F32R, tag="wv")
        hT = p_wq.tile([P, DC, SEQ], F32R, tag="hT")

        # ---------- stage 1: embedding gather + positional + transpose ----------
        # interleave gather_j / pos_j DMA issue so chunk j's inputs land together
        # NOTE: multi-offset indirect DMA (several offsets per partition)
        # returns wrong data on real hw — keep one gather per 128-token chunk.
        hb_ts = []
        pos_ts = []
        for jj in range(NT):
            hb_t = p_wq.tile([P, D], BF16, tag="hb", bufs=NT, name=f"hb{jj}")
            nc.gpsimd.indirect_dma_start(
                out=hb_t[:],
                out_offset=None,
                in_=emb_d[:],
                in_offset=bass.IndirectOffsetOnAxis(ap=x_sb[:, jj : jj + 1], axis=0),
            )
            hb_ts.append(hb_t)
            pos_t = p_wq.tile([P, D], BF16, tag="pos", bufs=NT, name=f"pos{jj}")
            nc.sync.dma_start(out=pos_t[:], in_=pos_d[jj * P : (jj + 1) * P, :])
            pos_ts.append(pos_t)

        make_identity(nc, ident_f[:])
        nc.vector.tensor_copy(out=ident[:], in_=ident_f[:])
        nc.gpsimd.tensor_copy(out=ident_b[:], in_=ident_f[:])

        def s1_add(j):
            nc.vector.tensor_add(out=h_sb[:, j, :], in0=hb_ts[j][:], in1=pos_ts[j][:])

        def s1_trans(j):
            pt = psum_t.tile([P, 512], F32, tag="pt", name=f"s1pt{j}")
            for m in range(DC):
                nc.tensor.transpose(
                    out=_r(pt[:, m * P : (m + 1) * P]),
                    in_=_r(h_sb[:, j, m * P : (m + 1) * P]),
                    identity=_r(ident[:]),
                )
            nc.scalar.copy(out=hT[:, :, j * P : (j + 1) * P], in_=pt[:])

        for k in range(NT + 1):
            if k < NT:
                s1_add(k)
            if k >= 1:
                s1_trans(k - 1)

        # weight DMAs issued after stage-1 loads so embeddings/pos win the queue
        for w_sb, w_d in ((wq_sb, wq_d), (wk_sb, wk_d), (wv_sb, wv_d), (wo_sb, wo_d)):
            nc.sync.dma_start(out=w_sb[:], in_=_r(w_d[:].rearrange("(k p) o -> p k o", p=P)))

        # ---------- stage 2: qT / kT (d-major), v (token-major) ----------
        # t-major order: all groups needing hT[0:512] first (PE is in-order)
        for t in range(SEQ // 512):
            for (wt, bt, dst) in ((wq_sb, bq_sb, qT), (wk_sb, bk_sb, kT)):
                for m in range(DC):
                    ps = psum.tile([P, 512], F32, tag="ps")
                    for ki in range(DC):
                        nc.tensor.matmul(
                            ps[:],
                            _r(wt[:, ki, m * P : (m + 1) * P]),
                            _r(hT[:, ki, t * 512 : (t + 1) * 512]),
                            start=(ki == 0),
                            stop=(ki == DC - 1),
                        )
                    dslc = dst[:, m, t * 512 : (t + 1) * 512]
                    if bt is not None:
                        nc.scalar.activation(
                            out=dslc, in_=ps[:], func=AF.Identity,
                            bias=bt[:, m : m + 1], scale=1.0,
                        )
                    elif dst is kT:
                        nc.vector.tensor_copy(out=dslc, in_=ps[:])
                    else:
                        nc.scalar.copy(out=dslc, in_=ps[:])
            for j in range(4 * t, 4 * t + 4):
                ps = psum.tile([P, 512], F32, tag="ps")
                for ki in range(DC):
                    nc.tensor.matmul(
                        ps[:],
                        _r(hT[:, ki, j * P : (j + 1) * P]),
                        _r(wv_sb[:, ki, :]),
                        start=(ki == 0),
                        stop=(ki == DC - 1),
                    )
                if bv_bc is not None:
                    nc.vector.tensor_add(out=v_sb[:, j, :], in0=ps[:], in1=bv_bc[:])
                else:
                    nc.vector.tensor_copy(out=v_sb[:, j, :], in_=ps[:])

        p_wq.release()

        # ---------- stage 3 wave 1: scores + softmax ----------
        attns = []
        recips = []
        for i in range(NT):
            ws = _window_start(i)
            nw = min(512, max(256, (i + 1) * P))  # live window (>=256 keeps f32r fast)
            ps_s = psum.tile([P, 512], F32, tag="ps")
            for ki in range(DC):
                nc.tensor.matmul(
                    ps_s[:, :nw],
                    _r(qT[:, ki, i * P : (i + 1) * P]),
                    _r(kT[:, ki, ws : ws + nw]),
                    start=(ki == 0),
                    stop=(ki == DC - 1),
                )
            msk_t = p_at.tile([P, 512], BF16, tag="msk", bufs=4)
            nc.sync.dma_start(out=msk_t[:], in_=msk_d[i])
            s_t = p_at.tile([P, 512], F32, tag="s_t", bufs=2)
            nc.vector.tensor_add(out=s_t[:, :nw], in0=ps_s[:, :nw], in1=msk_t[:, :nw])
            attn = p_at.tile([P, 512], BF16, tag="attn", bufs=NT, name=f"attn{i}")
            denom = small.tile([P, 1], F32, tag="denom")
            nc.scalar.activation(
                out=attn[:, :nw], in_=s_t[:, :nw], func=AF.Exp,
                bias=0.0, scale=INV_SQRT_D,
                accum_out=denom[:, 0:1],
            )
            recip = small.tile([P, 1], F32, tag="recip", bufs=NT, name=f"recip{i}")
            nc.vector.reciprocal(out=recip[:], in_=denom[:])
            attns.append(attn)
            recips.append(recip)

        p_qk.release()

        # ----- right-side pools for FFN phase -----
        whpool = tc.alloc_tile_pool(name="whstream", bufs=8, side="right")
        p_h1 = tc.alloc_tile_pool(name="h1p", bufs=1, side="right")
        h1_sb = p_h1.tile([P, NT, D], F32R, tag="h1")
        h1T = p_h1.tile([P, DC, SEQ], BF16, tag="h1T")
        w1_sb = p_h1.tile([P, DC, DFF], BF16, tag="w1")
        nc.sync.dma_start(out=w1_sb[:], in_=w1_d[:].rearrange("(k p) o -> p k o", p=P))
        w2_sb = p_h1.tile([P, FC, D], BF16, tag="w2")
        nc.sync.dma_start(out=w2_sb[:], in_=w2_d[:].rearrange("(k p) o -> p k o", p=P))

        # ---------- stage 3 wave 2 + stage 4, software-pipelined ----------
        p_st4 = tc.alloc_tile_pool(name="st4", bufs=3)
        attnTs = [None] * NT
        o_ts = [None] * NT

        def w2_a(i):  # attn transposes + attnT eviction
            ws = _window_start(i)
            kb0 = ws // P
            nkb = min(DC, i - kb0 + 1)
            pt = psum_t.tile([P, 512], BF16, tag="ptb", bufs=1, name=f"atp{i}")
            for kk in range(nkb):
                nc.tensor.transpose(
                    out=pt[:, kk * P : (kk + 1) * P],
                    in_=attns[i][:, kk * P : (kk + 1) * P],
                    identity=ident_b[:],
                )
            attnT = p_at.tile([P, 512], BF16, tag="attnT", bufs=3, name=f"attnT{i}")
            nc.vector.tensor_copy(out=attnT[:, : nkb * P], in_=pt[:, : nkb * P])
            attnTs[i] = attnT

        def w2_b(i):  # o matmuls + scale
            ws = _window_start(i)
            kb0 = ws // P
            nkb = min(DC, i - kb0 + 1)
            ps_o = psum.tile([P, 512], F32, tag="ps", name=f"pso{i}")
            for kk in range(nkb):
                nc.tensor.matmul(
                    ps_o[:],
                    attnTs[i][:, kk * P : (kk + 1) * P],
                    v_sb[:, kb0 + kk, :],
                    start=(kk == 0),
                    stop=(kk == nkb - 1),
                )
            o_t = p_at.tile([P, D], F32R, tag="o_t", bufs=3, name=f"o_t{i}")
            nc.vector.tensor_scalar_mul(out=o_t[:], in0=ps_o[:], scalar1=recips[i][:, 0:1])
            o_ts[i] = o_t

        def w2_c(i):  # oT transposes + eviction
            pt2 = psum_t.tile([P, 512], F32, tag="pt", name=f"otp{i}")
            for m in range(DC):
                nc.tensor.transpose(
                    out=_r(pt2[:, m * P : (m + 1) * P]),
                    in_=_r(o_ts[i][:, m * P : (m + 1) * P]),
                    identity=_r(ident[:]),
                )
            nc.vector.tensor_copy(out=oT[:, :, i * P : (i + 1) * P], in_=pt2[:])

        def s4_proj(j):  # attn projection + residual + LN1 (no transpose)
            ps = psum.tile([P, 512], F32, tag="ps", name=f"psp{j}")
            for m in range(DC):
                nc.tensor.matmul(
                    ps[:],
                    _r(oT[:, m, j * P : (j + 1) * P]),
                    _r(wo_sb[:, m, :]),
                    start=(m == 0),
                    stop=(m == DC - 1),
                )
            r1 = p_st4.tile([P, D], F32, tag="r1", name=f"r1_{j}")
            nc.vector.tensor_add(out=r1[:], in0=h_sb[:, j, :], in1=ps[:])
            if bo_bc is not None:
                nc.vector.tensor_add(out=r1[:], in0=r1[:], in1=bo_bc[:])
            stats = small.tile([P, 6], F32, tag="stats")
            nc.vector.bn_stats(out=stats[:], in_=r1[:])
            mv = small.tile([P, 2], F32, tag="mv")
            nc.vector.bn_aggr(out=mv[:], in_=stats[:])
            stdt = small.tile([P, 1], F32, tag="stdt")
            nc.scalar.activation(
                out=stdt[:], in_=mv[:, 1:2], func=AF.Sqrt,
                bias=eps_t[:, 0:1], scale=1.0,
            )
            rstd = small.tile([P, 1], F32, tag="rstd")
            nc.vector.reciprocal(out=rstd[:], in_=stdt[:])
            nc.vector.tensor_scalar(
                out=h1_sb[:, j, :], in0=r1[:],
                scalar1=mv[:, 0:1], scalar2=rstd[:, 0:1],
                op0=mybir.AluOpType.subtract, op1=mybir.AluOpType.mult,
            )
            if g1_bc is not None:
                nc.vector.tensor_mul(out=h1_sb[:, j, :], in0=h1_sb[:, j, :], in1=g1_bc[:])
            if be1_bc is not None:
                nc.vector.tensor_add(out=h1_sb[:, j, :], in0=h1_sb[:, j, :], in1=be1_bc[:])

        def s4_trans(j):  # h1 transposes + h1T eviction
            pt3 = psum_t.tile([P, 512], F32, tag="pt", name=f"h1p{j}")
            for m in range(DC):
                nc.tensor.transpose(
                    out=_r(pt3[:, m * P : (m + 1) * P]),
                    in_=_r(h1_sb[:, j, m * P : (m + 1) * P]),
                    identity=_r(ident[:]),
                )
            nc.scalar.copy(out=h1T[:, :, j * P : (j + 1) * P], in_=pt3[:])

        for k in range(NT + 4):
            if k < NT:
                w2_a(k)
            if 1 <= k < NT + 1:
                w2_b(k - 1)
            if 2 <= k < NT + 2:
                w2_c(k - 2)
            if 3 <= k < NT + 3:
                s4_proj(k - 3)
            if 4 <= k:
                s4_trans(k - 4)

        p_st4.release()
        p_at.release()
        p_v.release()
        p_oT.release()
        p_woh.release()

        # ---------- stage 5: FFN up, f1T = relu(W1^T @ h1T + b1) ----------
        p_f1 = tc.alloc_tile_pool(name="f1p", bufs=1, side="right")
        f1T = p_f1.tile([P, FC, SEQ], BF16, tag="f1T")
        def ffn1_group(n, t):
            ps = psum.tile([P, 512], F32, tag="ps", name=f"psf{n}_{t}")
            for ki in range(DC):
                nc.tensor.matmul(
                    ps[:],
                    w1_sb[:, ki, n * P : (n + 1) * P],
                    h1T[:, ki, t * 512 : (t + 1) * 512],
                    start=(ki == 0),
                    stop=(ki == DC - 1),
                )
            fslc = f1T[:, n, t * 512 : (t + 1) * 512]
            if b1_sb is not None:
                nc.vector.tensor_scalar(
                    out=fslc, in0=ps[:],
                    scalar1=b1_sb[:, n : n + 1], scalar2=0.0,
                    op0=mybir.AluOpType.add, op1=mybir.AluOpType.max,
                )
            else:
                nc.vector.tensor_scalar_max(out=fslc, in0=ps[:], scalar1=0.0)

        # ---------- stage 6: FFN down + residual + LN2 (pipelined) ----------
        def s6_main(j):
            ps = psum.tile([P, 512], F32, tag="ps", name=f"ps6_{j}")
            for n in range(FC):
                nc.tensor.matmul(
                    ps[:],
                    f1T[:, n, j * P : (j + 1) * P],
                    w2_sb[:, n, :],
                    start=(n == 0),
                    stop=(n == FC - 1),
                )
            r2 = p_f1.tile([P, D], F32, tag="r2", bufs=3, name=f"r2_{j}")
            nc.vector.tensor_add(out=r2[:], in0=h1_sb[:, j, :], in1=ps[:])
            if b2_bc is not None:
                nc.vector.tensor_add(out=r2[:], in0=r2[:], in1=b2_bc[:])
            stats = small.tile([P, 6], F32, tag="stats")
            nc.vector.bn_stats(out=stats[:], in_=r2[:])
            mv = small.tile([P, 2], F32, tag="mv")
            nc.vector.bn_aggr(out=mv[:], in_=stats[:])
            stdt = small.tile([P, 1], F32, tag="stdt")
            nc.scalar.activation(
                out=stdt[:], in_=mv[:, 1:2], func=AF.Sqrt,
                bias=eps_t[:, 0:1], scale=1.0,
            )
            rstd = small.tile([P, 1], F32, tag="rstd")
            nc.vector.reciprocal(out=rstd[:], in_=stdt[:])
            h2_t = p_f1.tile([P, D], F32R, tag="h2_t", bufs=3, name=f"h2t_{j}")
            nc.vector.tensor_scalar(
                out=h2_t[:], in0=r2[:],
                scalar1=mv[:, 0:1], scalar2=rstd[:, 0:1],
                op0=mybir.AluOpType.subtract, op1=mybir.AluOpType.mult,
            )
            if g2_bc is not None:
                nc.vector.tensor_mul(out=h2_t[:], in0=h2_t[:], in1=g2_bc[:])
            if be2_bc is not None:
                nc.vector.tensor_add(out=h2_t[:], in0=h2_t[:], in1=be2_bc[:])
            return h2_t

        h2ts = [None] * NT

        def s6_trans(j):
            # transpose h2, then split-quantize to fp8: a8T = fp8(h2T),
            # da8T = fp8(h2T - a8T)
            pt = psum_t.tile([P, DC, P], F32, tag="pt", name=f"h2p{j}")
            for m in range(DC):
                nc.tensor.transpose(
                    out=_r(pt[:, m, :]),
                    in_=_r(h2ts[j][:, m * P : (m + 1) * P]),
                    identity=_r(ident[:]),
                )
            nc.scalar.copy(out=a8T[j][:, :, :], in_=pt[:, :, :])
            da_t = p_f1.tile([P, DC, P], F32, tag="da_t", bufs=2, name=f"da_t{j}")
            nc.vector.tensor_sub(out=da_t[:, :, :], in0=pt[:, :, :], in1=a8T[j][:, :, :])
            nc.gpsimd.tensor_copy(out=da8T[j][:, :, :], in_=da_t[:, :, :])

        # head chunks for vc=0,1 interleaved into stage-6 so PE fills LN waits
        def load_whv(vc, nm):
            whv = whpool.tile([P, 2, 2, 2, 256], FP8, tag="whv", name=f"whv{nm}")
            nc.sync.dma_start(out=whv[:], in_=wh8_d[:, vc])
            dwv = whpool.tile([P, 2, 2, 2, 256], FP8, tag="dwv", name=f"dwv{nm}")
            nc.sync.dma_start(out=dwv[:], in_=dwh8_d[:, vc])
            return whv, dwv

        whv0, dwv0 = load_whv(0, "0")
        otile0 = opool.tile([P, NT, 512], BF16 if OUT_BF16 else F32, tag="ot", name="otile0")
        whv1, dwv1 = load_whv(1, "1")
        otile1 = opool.tile([P, NT, 512], BF16 if OUT_BF16 else F32, tag="ot", name="otile1")

        def head_j(whv, dwv, otile, j, toggle):
            # 3-pass fp8 DoubleRow: a@w + a@dw + da@w, one PSUM group per
            # 256-vocab half; scale 1/WH_SCALE folded into the eviction
            ps = psum.tile([P, 512], F32, tag="ps", name=f"psh{toggle}_{j}")
            for t in range(2):
                ops = []
                for ki2 in range(2):
                    lhs_a = a8T[j][:, 2 * ki2 : 2 * ki2 + 2, :]
                    lhs_da = da8T[j][:, 2 * ki2 : 2 * ki2 + 2, :]
                    ops.append((lhs_a, whv[:, t, ki2]))
                    ops.append((lhs_a, dwv[:, t, ki2]))
                    ops.append((lhs_da, whv[:, t, ki2]))
                for n, (l, r) in enumerate(ops):
                    nc.tensor.matmul(
                        ps[:, t * 256 : (t + 1) * 256],
                        l,
                        r,
                        start=(n == 0),
                        stop=(n == len(ops) - 1),
                        perf_mode=DR,
                    )
            if bh_sb_for(toggle) is not None:
                nc.vector.tensor_scalar_mul(
                    out=otile[:, j, :], in0=ps[:], scalar1=1.0 / WH_SCALE
                )
                nc.vector.tensor_add(
                    out=otile[:, j, :], in0=otile[:, j, :], in1=bh_sb_for(toggle)[:]
                )
            elif j % 2 == 0:
                nc.vector.tensor_scalar_mul(
                    out=otile[:, j, :], in0=ps[:], scalar1=1.0 / WH_SCALE
                )
            else:
                nc.scalar.activation(
                    out=otile[:, j, :], in_=ps[:], func=AF.Identity,
                    bias=0.0, scale=1.0 / WH_SCALE,
                )

        _bh_tiles = {}

        def bh_sb_for(key):
            return _bh_tiles.get(key)

        if bh_d is not None:
            bh0 = whpool.tile([P, 512], F32, tag="bh", bufs=2, name="bh0")
            nc.sync.dma_start(out=bh0[:], in_=_bcast_ap(bh_d[0:512]))
            _bh_tiles[0] = bh0
            bh1 = whpool.tile([P, 512], F32, tag="bh", bufs=2, name="bh1")
            nc.sync.dma_start(out=bh1[:], in_=_bcast_ap(bh_d[512:1024]))
            _bh_tiles[1] = bh1

        for t in range(SEQ // 512):
            for n in range(FC):
                ffn1_group(n, t)
                if t == 1 and n % 2 == 1:
                    j = n // 2
                    h2ts[j] = s6_main(j)

        for k in range(NT + 3):
            if 4 <= k < NT:
                h2ts[k] = s6_main(k)
            if 1 <= k <= NT:
                s6_trans(k - 1)
            if 2 <= k <= NT + 1:
                head_j(whv0, dwv0, otile0, k - 2, 0)
            if 3 <= k <= NT + 2:
                head_j(whv1, dwv1, otile1, k - 3, 1)
        out_rr = out_d[:].rearrange("(j p) v -> p j v", p=P)
        nc.sync.dma_start(out=out_rr[:, :, 0:512], in_=otile0[:])
        nc.sync.dma_start(out=out_rr[:, :, 512:1024], in_=otile1[:])

        p_f1.release()
        p_h1.release()

        # ---------- stage 7: vocab head (vc >= 2) ----------
        out_r = out_d[:].rearrange("(j p) v -> p j v", p=P)
        for vc in range(2, NV):
            whv, dwv = load_whv(vc, str(vc))
            if bh_d is not None:
                bh_bc = whpool.tile([P, 512], F32, tag="bh", bufs=2, name=f"bh{vc}")
                nc.sync.dma_start(
                    out=bh_bc[:], in_=_bcast_ap(bh_d[vc * 512 : (vc + 1) * 512])
                )
                _bh_tiles[vc] = bh_bc
            otile = opool.tile([P, NT, 512], BF16 if OUT_BF16 else F32, tag="ot")
            vs = slice(vc * 512, (vc + 1) * 512)
            # split stores per j-half (last chunk: per j-pair) to shrink the
            # final DMA drain after the last matmul
            if vc == NV - 1:
                for j in range(NT):
                    head_j(whv, dwv, otile, j, vc)
                    if j % 2 == 1:
                        nc.sync.dma_start(
                            out=out_r[:, j - 1 : j + 1, vs],
                            in_=otile[:, j - 1 : j + 1, :],
                        )
            else:
                for j in range(NT):
                    head_j(whv, dwv, otile, j, vc)
                    if j == NT // 2 - 1 or j == NT - 1:
                        h0 = j + 1 - NT // 2
                        nc.sync.dma_start(
                            out=out_r[:, h0 : j + 1, vs],
                            in_=otile[:, h0 : j + 1, :],
                        )

        whpool.release()
        p_h2T.release()
        opool.release()
        psum_t.release()
        psum.release()
        small.release()
        const.release()

    nc.finalize()
    return nc


_PROGRAM_CACHE: dict = {}


def _get_program(flags: dict) -> bass.Bass:
    key = tuple(sorted(flags.items()))
    if key not in _PROGRAM_CACHE:
        _PROGRAM_CACHE[key] = _build_program(flags)
    return _PROGRAM_CACHE[key]


def _prep(x, embed_tab, row_embed, col_embed, Wq, bq, Wk, bk, Wv, bv, Wo, bo,
          ln1_g, ln1_b, W1, b1, W2, b2, ln2_g, ln2_b, Wh, bh):
    """Shared host-side prep: flags, common input map, per-core x shards."""
    f32c = lambda a: np.ascontiguousarray(np.asarray(a, dtype=np.float32))
    x = np.asarray(x)
    B = x.shape[0]
    assert x.shape == (B, SEQ)

    import ml_dtypes

    bf16 = ml_dtypes.bfloat16
    fp8 = ml_dtypes.float8_e4m3
    arrs = dict(
        wq=f32c(Wq), wk=f32c(Wk), wv=f32c(Wv), wo=f32c(Wo),
    )
    arrs["emb"] = np.ascontiguousarray(f32c(embed_tab).astype(bf16))
    arrs["w1"] = np.ascontiguousarray(f32c(W1).astype(bf16))
    arrs["w2"] = np.ascontiguousarray(f32c(W2).astype(bf16))
    # fp8 split head weights, pre-scaled by WH_SCALE:
    #   wh8 = fp8(Wh*S), dwh8 = fp8(Wh*S - wh8); layout [p, vc, t, ki2, i, n]
    whs = f32c(Wh) * WH_SCALE
    w8 = whs.astype(fp8)
    dw8 = (whs - w8.astype(np.float32)).astype(fp8)

    def _wh_layout(a):
        # [D=512, V] -> [ki2, i, p, vc, t, n] -> [p, vc, t, ki2, i, n]
        a = a.reshape(2, 2, P, NV, 2, 256)
        return np.ascontiguousarray(a.transpose(2, 3, 4, 0, 1, 5))

    arrs["wh8"] = _wh_layout(w8)
    arrs["dwh8"] = _wh_layout(dw8)
    pos = np.concatenate(
        [np.repeat(f32c(row_embed), GW, axis=0), np.tile(f32c(col_embed), (GH, 1))],
        axis=-1,
    )
    arrs["pos"] = np.ascontiguousarray(pos.astype(bf16))
    arrs["maskt"] = np.ascontiguousarray(_mask_tiles().astype(bf16))

    bias_map = dict(
        bq=f32c(bq), bk=f32c(bk), bv=f32c(bv), bo=f32c(bo), b1=f32c(b1),
        b2=f32c(b2), bh=f32c(bh), be1=f32c(ln1_b), be2=f32c(ln2_b),
    )
    gain_map = dict(g1=f32c(ln1_g), g2=f32c(ln2_g))
    flags = {k: bool(np.any(v)) for k, v in bias_map.items()}
    flags.update({k: bool(np.any(v != 1.0)) for k, v in gain_map.items()})
    for k, v in {**bias_map, **gain_map}.items():
        if flags[k]:
            arrs[k] = v

    xs = [np.ascontiguousarray(x[c].astype(np.int32)) for c in range(B)]
    return flags, arrs, xs, B


def kernel(**inputs):
    flags, arrs, xs, B = _prep(**inputs)
    nc = _get_program(flags)
    core_ids = list(range(8))
    in_maps = [{**arrs, "x": xs[c % B]} for c in core_ids]
    res = run_bass_kernel_spmd(nc, in_maps, core_ids)
    out = np.stack([res.results[c]["out"] for c in range(B)], axis=0)
    return np.asarray(out, dtype=np.float32)



# revision 47
# speedup vs baseline: 1.0543x; 1.0261x over previous
"""Trainium2 Bass kernel for a small autoregressive transformer block with
local-windowed causal attention and a large (16k) vocab head.

Data-parallel over batch: batch item b runs on NeuronCore b (8 cores).
Per core:
  h   = embed_tab[x] + pos                      [1024, 512]
  q/k/v = h @ Wq/k/v (+b)                       [1024, 512]
  s   = q @ k^T / sqrt(D) + local_causal_mask   (banded, window <= 298)
  o   = softmax(s) @ v @ Wo (+bo)
  h1  = LN(h + o);  f = relu(h1@W1+b1)@W2+b2;  h2 = LN(h1 + f)
  out = h2 @ Wh (+bh)                           [1024, 16384]

Body matmuls run as float32r (full-rate fp32 with N=512 moving dim). The
vocab head runs as fp8e4m3 DoubleRow (256-deep contraction at 0.5 cyc/row)
with a 3-pass residual split (a@w + a@dw + da@w, Wh pre-scaled by 64) to
stay within the 2e-2 error budget.
kernel(**inputs) takes full unsharded inputs, returns [8, 1024, 16384] f32.
"""

import math
import numpy as np

import concourse.bass as bass
import concourse.mybir as mybir
import concourse.tile as tile
from concourse import bacc
from concourse.bass_utils import run_bass_kernel_spmd
from concourse.masks import make_identity

# ---- problem constants (hardcoded per contract) ----
GH = 32
GW = 32
SEQ = 1024
WIN = 9
D = 512
DFF = 1024
VOCAB = 16384
EPS = 1e-5
NEG = -1e30

P = 128
NT = SEQ // P        # 8 token chunks
DC = D // P          # 4 d chunks
FC = DFF // P        # 8 dff chunks
NV = VOCAB // 512    # 32 vocab chunks
INV_SQRT_D = 1.0 / math.sqrt(D)

F32 = mybir.dt.float32
F32R = mybir.dt.float32r
BF16 = mybir.dt.bfloat16
FP8 = mybir.dt.float8e4
I32 = mybir.dt.int32
OUT_BF16 = True
WH_SCALE = 64.0  # Wh pre-scaled by 64 on host; folded out at PSUM eviction
DR = mybir.MatmulPerfMode.DoubleRow
AF = mybir.ActivationFunctionType


def _window_start(i: int) -> int:
    # k-window [ws, ws+512) covers all allowed keys for query chunk i
    # (max lookback is WIN*GW + WIN = 297 < 384).
    return 128 * max(0, i - 3)


def _mask_tiles() -> np.ndarray:
    idx = np.arange(SEQ)
    r, c = idx // GW, idx % GW
    allow = (
        (np.abs(r[:, None] - r[None, :]) <= WIN)
        & (np.abs(c[:, None] - c[None, :]) <= WIN)
        & (idx[None, :] <= idx[:, None])
    )
    maskf = np.where(allow, 0.0, NEG).astype(np.float32)
    tiles = np.empty((NT, P, 512), np.float32)
    for i in range(NT):
        ws = _window_start(i)
        tiles[i] = maskf[i * P : (i + 1) * P, ws : ws + 512]
    return tiles


def _r(ap):
    """bitcast to float32r for full-rate fp32 matmul."""
    return ap.bitcast(F32R)


def _bcast_ap(a: bass.AP) -> bass.AP:
    """[n] DRAM vector AP -> [P, n] partition-broadcast DMA source."""
    return bass.AP(tensor=a.tensor, offset=a.offset, ap=[[0, P], *a.ap])


def _build_program(flags: dict, wh_bufs: int = 6, msk_bufs: int = 6, lean: bool = False) -> bass.Bass:
    nc = bacc.Bacc("TRN2", target_bir_lowering=False)

    # ---------- I/O ----------
    x_d = nc.declare_dram_parameter("x", [SEQ], I32, False)
    emb_d = nc.declare_dram_parameter("emb", [VOCAB, D], BF16, False)
    pos_d = nc.declare_dram_parameter("pos", [SEQ, D], BF16, False)
    msk_d = nc.declare_dram_parameter("maskt", [NT, P, 512], BF16, False)
    wq_d = nc.declare_dram_parameter("wq", [D, D], BF16, False)
    wk_d = nc.declare_dram_parameter("wk", [D, D], BF16, False)
    wv_d = nc.declare_dram_parameter("wv", [D, D], BF16, False)
    wo_d = nc.declare_dram_parameter("wo", [D, D], F32, False)
    w1_d = nc.declare_dram_parameter("w1", [D, DFF], BF16, False)
    w2_d = nc.declare_dram_parameter("w2", [DFF, D], BF16, False)
    # fp8 head weights: [p, vc, vhalf, ki2, i, 256] with d = 128*(2*ki2+i)+p
    wh8_d = nc.declare_dram_parameter("wh8", [P, NV, 2, 2, 2, 256], FP8, False)
    dwh8_d = nc.declare_dram_parameter("dwh8", [P, NV, 2, 2, 2, 256], FP8, False)
    dp = lambda name, shape: nc.declare_dram_parameter(name, shape, F32, False)
    bq_d = dp("bq", [D]) if flags["bq"] else None
    bk_d = dp("bk", [D]) if flags["bk"] else None
    bv_d = dp("bv", [D]) if flags["bv"] else None
    bo_d = dp("bo", [D]) if flags["bo"] else None
    b1_d = dp("b1", [DFF]) if flags["b1"] else None
    b2_d = dp("b2", [D]) if flags["b2"] else None
    bh_d = dp("bh", [VOCAB]) if flags["bh"] else None
    g1_d = dp("g1", [D]) if flags["g1"] else None
    be1_d = dp("be1", [D]) if flags["be1"] else None
    g2_d = dp("g2", [D]) if flags["g2"] else None
    be2_d = dp("be2", [D]) if flags["be2"] else None
    out_d = nc.declare_dram_parameter("out", [SEQ, VOCAB], BF16 if OUT_BF16 else F32, True)

    with tile.TileContext(nc) as tc:
        # ----- whole-kernel pools -----
        const = tc.alloc_tile_pool(name="const", bufs=1)
        small = tc.alloc_tile_pool(name="small", bufs=8)
        psum = tc.alloc_tile_pool(name="psA", bufs=5, space="PSUM")
        psum_t = tc.alloc_tile_pool(name="psT", bufs=2, space="PSUM")
        opool = tc.alloc_tile_pool(name="outev", bufs=3, side="right")
        p_h2T = tc.alloc_tile_pool(name="h2Tp", bufs=1, side="right")

        ident_f = const.tile([P, P], F32, tag="ident_f")
        ident = const.tile([P, P], F32R, tag="ident")
        ident_b = const.tile([P, P], BF16, tag="ident_b")
        eps_t = const.tile([P, 1], F32, tag="eps")
        nc.vector.memset(eps_t[:], EPS)
        x_sb = const.tile([P, NT], I32, tag="x_sb")
        nc.sync.dma_start(out=x_sb[:], in_=x_d[:].rearrange("(j p) -> p j", p=P))

        def load_col_bias(handle, nchunks, tag):
            # [nchunks*P] DRAM -> [P, nchunks] (chunk m in column m)
            t = const.tile([P, nchunks], F32, tag=tag)
            nc.sync.dma_start(out=t[:], in_=handle[:].rearrange("(m p) -> p m", p=P))
            return t

        def load_bcast(handle, n, tag):
            t = const.tile([P, n], F32, tag=tag)
            nc.sync.dma_start(out=t[:], in_=_bcast_ap(handle[:]))
            return t

        bq_sb = load_col_bias(bq_d, DC, "bq") if bq_d else None
        bk_sb = load_col_bias(bk_d, DC, "bk") if bk_d else None
        b1_sb = load_col_bias(b1_d, FC, "b1") if b1_d else None
        bv_bc = load_bcast(bv_d, D, "bv") if bv_d else None
        bo_bc = load_bcast(bo_d, D, "bo") if bo_d else None
        b2_bc = load_bcast(b2_d, D, "b2") if b2_d else None
        g1_bc = load_bcast(g1_d, D, "g1") if g1_d else None
        be1_bc = load_bcast(be1_d, D, "be1") if be1_d else None
        g2_bc = load_bcast(g2_d, D, "g2") if g2_d else None
        be2_bc = load_bcast(be2_d, D, "be2") if be2_d else None

        a8T = [p_h2T.tile([P, DC, P], FP8, tag=f"a8T{j}", name=f"a8T{j}") for j in range(NT)]
        da8T = [p_h2T.tile([P, DC, P], FP8, tag=f"da8T{j}", name=f"da8T{j}") for j in range(NT)]

        # ----- phase A pools (left, LIFO) -----
        p_woh = tc.alloc_tile_pool(name="woh", bufs=1)         # wo, h  (-> stage 4)
        wo_sb = p_woh.tile([P, DC, D], F32R, tag="wo")
        h_sb = p_woh.tile([P, NT, D], F32R, tag="h")

        p_oT = tc.alloc_tile_pool(name="oTp", bufs=1)          # oT    (-> stage 4)
        oT = p_oT.tile([P, DC, SEQ], F32R, tag="oT")

        p_v = tc.alloc_tile_pool(name="vp", bufs=1)            # v (-> wave 2)
        v_sb = p_v.tile([P, NT, D], BF16, tag="v")
        p_at = tc.alloc_tile_pool(name="attnw", bufs=3)        # softmax work (-> stage 4)
        p_qk = tc.alloc_tile_pool(name="qkp", bufs=1)          # qT,kT (-> wave 1)
        qT = p_qk.tile([P, DC, SEQ], F32R, tag="qT")
        kT = p_qk.tile([P, DC, SEQ], F32R, tag="kT")

        p_wq = tc.alloc_tile_pool(name="wqp", bufs=1)          # wq,wk,wv,hT (-> stage 2)
        wq_sb = p_wq.tile([P, DC, D], BF16, tag="wq")
        wk_sb = p_wq.tile([P, DC, D], BF16, tag="wk")
        wv_sb = p_wq.tile([P, DC, D], BF16, tag="wv")
        hT = p_wq.tile([P, DC, SEQ], BF16, tag="hT")

        # ---------- stage 1: embedding gather + positional + transpose ----------
        # interleave gather_j / pos_j DMA issue so chunk j's inputs land together
        # NOTE: multi-offset indirect DMA (several offsets per partition)
        # returns wrong data on real hw — keep one gather per 128-token chunk.
        hb_ts = []
        pos_ts = []
        for jj in range(NT):
            hb_t = p_wq.tile([P, D], BF16, tag="hb", bufs=NT, name=f"hb{jj}")
            nc.gpsimd.indirect_dma_start(
                out=hb_t[:],
                out_offset=None,
                in_=emb_d[:],
                in_offset=bass.IndirectOffsetOnAxis(ap=x_sb[:, jj : jj + 1], axis=0),
            )
            hb_ts.append(hb_t)
            pos_t = p_wq.tile([P, D], BF16, tag="pos", bufs=NT, name=f"pos{jj}")
            nc.sync.dma_start(out=pos_t[:], in_=pos_d[jj * P : (jj + 1) * P, :])
            pos_ts.append(pos_t)
            if jj == 3:
                nc.sync.dma_start(
                    out=wq_sb[:], in_=wq_d[:].rearrange("(k p) o -> p k o", p=P)
                )
            elif jj == 5:
                nc.sync.dma_start(
                    out=wk_sb[:], in_=wk_d[:].rearrange("(k p) o -> p k o", p=P)
                )
            elif jj == 7:
                nc.sync.dma_start(
                    out=wv_sb[:], in_=wv_d[:].rearrange("(k p) o -> p k o", p=P)
                )

        make_identity(nc, ident_f[:])
        nc.vector.tensor_copy(out=ident[:], in_=ident_f[:])
        nc.gpsimd.tensor_copy(out=ident_b[:], in_=ident_f[:])

        def s1_add(j):
            nc.vector.tensor_add(out=h_sb[:, j, :], in0=hb_ts[j][:], in1=pos_ts[j][:])

        def s1_trans(j):
            pt = psum_t.tile([P, 512], F32, tag="pt", name=f"s1pt{j}")
            for m in range(DC):
                nc.tensor.transpose(
                    out=_r(pt[:, m * P : (m + 1) * P]),
                    in_=_r(h_sb[:, j, m * P : (m + 1) * P]),
                    identity=_r(ident[:]),
                )
            nc.scalar.copy(out=hT[:, :, j * P : (j + 1) * P], in_=pt[:])

        for k in range(5):
            if k < 4:
                s1_add(k)
            if k >= 1:
                s1_trans(k - 1)
        # adds for chunks 4-7 issued now so DVE never blocks them behind
        # stage-2 eviction work; their transposes run after stage-2 t=0
        for k in range(4, NT):
            s1_add(k)

        # wo load after the latency-critical stage-1 stream
        nc.sync.dma_start(
            out=wo_sb[:], in_=_r(wo_d[:].rearrange("(k p) o -> p k o", p=P))
        )

        # ---------- stage 2: qT / kT (d-major), v (token-major) ----------
        # t-major order: all groups needing hT[0:512] first (PE is in-order)
        def s2_qk(wt, bt, dst, t):
            for m in range(DC):
                ps = psum.tile([P, 512], F32, tag="ps")
                for ki in range(DC):
                    nc.tensor.matmul(
                        ps[:],
                        wt[:, ki, m * P : (m + 1) * P],
                        hT[:, ki, t * 512 : (t + 1) * 512],
                        start=(ki == 0),
                        stop=(ki == DC - 1),
                    )
                dslc = dst[:, m, t * 512 : (t + 1) * 512]
                if bt is not None:
                    nc.scalar.activation(
                        out=dslc, in_=ps[:], func=AF.Identity,
                        bias=bt[:, m : m + 1], scale=1.0,
                    )
                elif dst is kT:
                    nc.vector.tensor_copy(out=dslc, in_=ps[:])
                else:
                    nc.scalar.copy(out=dslc, in_=ps[:])

        def s2_v(t):
            for j in range(4 * t, 4 * t + 4):
                ps = psum.tile([P, 512], F32, tag="ps")
                for ki in range(DC):
                    nc.tensor.matmul(
                        ps[:],
                        hT[:, ki, j * P : (j + 1) * P],
                        wv_sb[:, ki, :],
                        start=(ki == 0),
                        stop=(ki == DC - 1),
                    )
                if bv_bc is not None:
                    nc.vector.tensor_add(out=v_sb[:, j, :], in0=ps[:], in1=bv_bc[:])
                else:
                    nc.vector.tensor_copy(out=v_sb[:, j, :], in_=ps[:])

        s2_qk(wq_sb, bq_sb, qT, 0)
        s2_qk(wk_sb, bk_sb, kT, 0)
        s2_v(0)
        for k in range(4, NT):
            s1_trans(k)
        s2_qk(wq_sb, bq_sb, qT, 1)
        s2_qk(wk_sb, bk_sb, kT, 1)
        s2_v(1)

        p_wq.release()

        # ---------- stage 3 wave 1: scores + softmax ----------
        attns = []
        recips = []
        for i in range(NT):
            ws = _window_start(i)
            nw = min(512, max(256, (i + 1) * P))  # live window (>=256 keeps f32r fast)
            ps_s = psum.tile([P, 512], F32, tag="ps")
            for ki in range(DC):
                nc.tensor.matmul(
                    ps_s[:, :nw],
                    _r(qT[:, ki, i * P : (i + 1) * P]),
                    _r(kT[:, ki, ws : ws + nw]),
                    start=(ki == 0),
                    stop=(ki == DC - 1),
                )
            msk_t = p_at.tile([P, 512], BF16, tag="msk", bufs=4)
            nc.sync.dma_start(out=msk_t[:], in_=msk_d[i])
            s_t = p_at.tile([P, 512], F32, tag="s_t", bufs=2)
            nc.vector.tensor_add(out=s_t[:, :nw], in0=ps_s[:, :nw], in1=msk_t[:, :nw])
            attn = p_at.tile([P, 512], BF16, tag="attn", bufs=NT, name=f"attn{i}")
            denom = small.tile([P, 1], F32, tag="denom")
            nc.scalar.activation(
                out=attn[:, :nw], in_=s_t[:, :nw], func=AF.Exp,
                bias=0.0, scale=INV_SQRT_D,
                accum_out=denom[:, 0:1],
            )
            recip = small.tile([P, 1], F32, tag="recip", bufs=NT, name=f"recip{i}")
            nc.vector.reciprocal(out=recip[:], in_=denom[:])
            attns.append(attn)
            recips.append(recip)

        p_qk.release()

        # ----- right-side pools for FFN phase -----
        whpool = tc.alloc_tile_pool(name="whstream", bufs=8, side="right")
        p_h1 = tc.alloc_tile_pool(name="h1p", bufs=1, side="right")
        h1_sb = p_h1.tile([P, NT, D], F32R, tag="h1")
        h1T = p_h1.tile([P, DC, SEQ], BF16, tag="h1T")
        w1_sb = p_h1.tile([P, DC, DFF], BF16, tag="w1")
        nc.sync.dma_start(out=w1_sb[:], in_=w1_d[:].rearrange("(k p) o -> p k o", p=P))
        w2_sb = p_h1.tile([P, FC, D], BF16, tag="w2")
        nc.sync.dma_start(out=w2_sb[:], in_=w2_d[:].rearrange("(k p) o -> p k o", p=P))

        # ---------- stage 3 wave 2 + stage 4, software-pipelined ----------
        p_st4 = tc.alloc_tile_pool(name="st4", bufs=3)
        attnTs = [None] * NT
        o_ts = [None] * NT

        def w2_a(i):  # attn transposes + attnT eviction
            ws = _window_start(i)
            kb0 = ws // P
            nkb = min(DC, i - kb0 + 1)
            pt = psum_t.tile([P, 512], BF16, tag="ptb", bufs=1, name=f"atp{i}")
            for kk in range(nkb):
                nc.tensor.transpose(
                    out=pt[:, kk * P : (kk + 1) * P],
                    in_=attns[i][:, kk * P : (kk + 1) * P],
                    identity=ident_b[:],
                )
            attnT = p_at.tile([P, 512], BF16, tag="attnT", bufs=3, name=f"attnT{i}")
            nc.vector.tensor_copy(out=attnT[:, : nkb * P], in_=pt[:, : nkb * P])
            attnTs[i] = attnT

        def w2_b(i):  # o matmuls + scale
            ws = _window_start(i)
            kb0 = ws // P
            nkb = min(DC, i - kb0 + 1)
            ps_o = psum.tile([P, 512], F32, tag="ps", name=f"pso{i}")
            for kk in range(nkb):
                nc.tensor.matmul(
                    ps_o[:],
                    attnTs[i][:, kk * P : (kk + 1) * P],
                    v_sb[:, kb0 + kk, :],
                    start=(kk == 0),
                    stop=(kk == nkb - 1),
                )
            o_t = p_at.tile([P, D], F32R, tag="o_t", bufs=3, name=f"o_t{i}")
            nc.vector.tensor_scalar_mul(out=o_t[:], in0=ps_o[:], scalar1=recips[i][:, 0:1])
            o_ts[i] = o_t

        def w2_c(i):  # oT transposes + eviction
            pt2 = psum_t.tile([P, 512], F32, tag="pt", name=f"otp{i}")
            for m in range(DC):
                nc.tensor.transpose(
                    out=_r(pt2[:, m * P : (m + 1) * P]),
                    in_=_r(o_ts[i][:, m * P : (m + 1) * P]),
                    identity=_r(ident[:]),
                )
            nc.vector.tensor_copy(out=oT[:, :, i * P : (i + 1) * P], in_=pt2[:])

        def s4_proj(j):  # attn projection + residual + LN1 (no transpose)
            ps = psum.tile([P, 512], F32, tag="ps", name=f"psp{j}")
            for m in range(DC):
                nc.tensor.matmul(
                    ps[:],
                    _r(oT[:, m, j * P : (j + 1) * P]),
                    _r(wo_sb[:, m, :]),
                    start=(m == 0),
                    stop=(m == DC - 1),
                )
            r1 = p_st4.tile([P, D], F32, tag="r1", name=f"r1_{j}")
            nc.vector.tensor_add(out=r1[:], in0=h_sb[:, j, :], in1=ps[:])
            if bo_bc is not None:
                nc.vector.tensor_add(out=r1[:], in0=r1[:], in1=bo_bc[:])
            stats = small.tile([P, 6], F32, tag="stats")
            nc.vector.bn_stats(out=stats[:], in_=r1[:])
            mv = small.tile([P, 2], F32, tag="mv")
            nc.vector.bn_aggr(out=mv[:], in_=stats[:])
            stdt = small.tile([P, 1], F32, tag="stdt")
            nc.scalar.activation(
                out=stdt[:], in_=mv[:, 1:2], func=AF.Sqrt,
                bias=eps_t[:, 0:1], scale=1.0,
            )
            rstd = small.tile([P, 1], F32, tag="rstd")
            nc.vector.reciprocal(out=rstd[:], in_=stdt[:])
            nc.vector.tensor_scalar(
                out=h1_sb[:, j, :], in0=r1[:],
                scalar1=mv[:, 0:1], scalar2=rstd[:, 0:1],
                op0=mybir.AluOpType.subtract, op1=mybir.AluOpType.mult,
            )
            if g1_bc is not None:
                nc.vector.tensor_mul(out=h1_sb[:, j, :], in0=h1_sb[:, j, :], in1=g1_bc[:])
            if be1_bc is not None:
                nc.vector.tensor_add(out=h1_sb[:, j, :], in0=h1_sb[:, j, :], in1=be1_bc[:])

        def s4_trans(j):  # h1 transposes + h1T eviction
            pt3 = psum_t.tile([P, 512], F32, tag="pt", name=f"h1p{j}")
            for m in range(DC):
                nc.tensor.transpose(
                    out=_r(pt3[:, m * P : (m + 1) * P]),
                    in_=_r(h1_sb[:, j, m * P : (m + 1) * P]),
                    identity=_r(ident[:]),
                )
            nc.scalar.copy(out=h1T[:, :, j * P : (j + 1) * P], in_=pt3[:])

        for k in range(NT + 4):
            if k < NT:
                w2_a(k)
            if 1 <= k < NT + 1:
                w2_b(k - 1)
            if 2 <= k < NT + 2:
                w2_c(k - 2)
            if 3 <= k < NT + 3:
                s4_proj(k - 3)
            if 4 <= k:
                s4_trans(k - 4)

        p_st4.release()
        p_at.release()
        p_v.release()
        p_oT.release()
        p_woh.release()

        # ---------- stage 5: FFN up, f1T = relu(W1^T @ h1T + b1) ----------
        p_f1 = tc.alloc_tile_pool(name="f1p", bufs=1, side="right")
        f1T = p_f1.tile([P, FC, SEQ], BF16, tag="f1T")
        def ffn1_group(n, t):
            ps = psum.tile([P, 512], F32, tag="ps", name=f"psf{n}_{t}")
            for ki in range(DC):
                nc.tensor.matmul(
                    ps[:],
                    w1_sb[:, ki, n * P : (n + 1) * P],
                    h1T[:, ki, t * 512 : (t + 1) * 512],
                    start=(ki == 0),
                    stop=(ki == DC - 1),
                )
            fslc = f1T[:, n, t * 512 : (t + 1) * 512]
            if b1_sb is not None:
                nc.vector.tensor_scalar(
                    out=fslc, in0=ps[:],
                    scalar1=b1_sb[:, n : n + 1], scalar2=0.0,
                    op0=mybir.AluOpType.add, op1=mybir.AluOpType.max,
                )
            else:
                nc.vector.tensor_scalar_max(out=fslc, in0=ps[:], scalar1=0.0)

        # ---------- stage 6: FFN down + residual + LN2 (pipelined) ----------
        def s6_main(j):
            ps = psum.tile([P, 512], F32, tag="ps", name=f"ps6_{j}")
            for n in range(FC):
                nc.tensor.matmul(
                    ps[:],
                    f1T[:, n, j * P : (j + 1) * P],
                    w2_sb[:, n, :],
                    start=(n == 0),
                    stop=(n == FC - 1),
                )
            r2 = p_f1.tile([P, D], F32, tag="r2", bufs=3, name=f"r2_{j}")
            nc.vector.tensor_add(out=r2[:], in0=h1_sb[:, j, :], in1=ps[:])
            if b2_bc is not None:
                nc.vector.tensor_add(out=r2[:], in0=r2[:], in1=b2_bc[:])
            stats = small.tile([P, 6], F32, tag="stats")
            nc.vector.bn_stats(out=stats[:], in_=r2[:])
            mv = small.tile([P, 2], F32, tag="mv")
            nc.vector.bn_aggr(out=mv[:], in_=stats[:])
            stdt = small.tile([P, 1], F32, tag="stdt")
            nc.scalar.activation(
                out=stdt[:], in_=mv[:, 1:2], func=AF.Sqrt,
                bias=eps_t[:, 0:1], scale=1.0,
            )
            rstd = small.tile([P, 1], F32, tag="rstd")
            nc.vector.reciprocal(out=rstd[:], in_=stdt[:])
            h2_t = p_f1.tile([P, D], F32R, tag="h2_t", bufs=3, name=f"h2t_{j}")
            nc.vector.tensor_scalar(
                out=h2_t[:], in0=r2[:],
                scalar1=mv[:, 0:1], scalar2=rstd[:, 0:1],
                op0=mybir.AluOpType.subtract, op1=mybir.AluOpType.mult,
            )
            if g2_bc is not None:
                nc.vector.tensor_mul(out=h2_t[:], in0=h2_t[:], in1=g2_bc[:])
            if be2_bc is not None:
                nc.vector.tensor_add(out=h2_t[:], in0=h2_t[:], in1=be2_bc[:])
            return h2_t

        h2ts = [None] * NT

        def s6_trans(j):
            # transpose h2, then split-quantize to fp8: a8T = fp8(h2T),
            # da8T = fp8(h2T - a8T)
            pt = psum_t.tile([P, DC, P], F32, tag="pt", name=f"h2p{j}")
            for m in range(DC):
                nc.tensor.transpose(
                    out=_r(pt[:, m, :]),
                    in_=_r(h2ts[j][:, m * P : (m + 1) * P]),
                    identity=_r(ident[:]),
                )
            nc.scalar.copy(out=a8T[j][:, :, :], in_=pt[:, :, :])
            da_t = p_f1.tile([P, DC, P], F32, tag="da_t", bufs=2, name=f"da_t{j}")
            nc.vector.tensor_sub(out=da_t[:, :, :], in0=pt[:, :, :], in1=a8T[j][:, :, :])
            nc.gpsimd.tensor_copy(out=da8T[j][:, :, :], in_=da_t[:, :, :])

        # head chunks for vc=0,1 interleaved into stage-6 so PE fills LN waits
        def load_whv(vc, nm):
            whv = whpool.tile([P, 2, 2, 2, 256], FP8, tag="whv", name=f"whv{nm}")
            nc.sync.dma_start(out=whv[:], in_=wh8_d[:, vc])
            dwv = whpool.tile([P, 2, 2, 2, 256], FP8, tag="dwv", name=f"dwv{nm}")
            nc.sync.dma_start(out=dwv[:], in_=dwh8_d[:, vc])
            return whv, dwv

        whv0, dwv0 = load_whv(0, "0")
        otile0 = opool.tile([P, NT, 512], BF16 if OUT_BF16 else F32, tag="ot", name="otile0")
        whv1, dwv1 = load_whv(1, "1")
        otile1 = opool.tile([P, NT, 512], BF16 if OUT_BF16 else F32, tag="ot", name="otile1")

        def head_j(whv, dwv, otile, j, toggle):
            # 3-pass fp8 DoubleRow: a@w + a@dw + da@w, one PSUM group per
            # 256-vocab half; scale 1/WH_SCALE folded into the eviction
            ps = psum.tile([P, 512], F32, tag="ps", name=f"psh{toggle}_{j}")
            for t in range(2):
                ops = []
                for ki2 in range(2):
                    lhs_a = a8T[j][:, 2 * ki2 : 2 * ki2 + 2, :]
                    lhs_da = da8T[j][:, 2 * ki2 : 2 * ki2 + 2, :]
                    ops.append((lhs_a, whv[:, t, ki2]))
                    ops.append((lhs_a, dwv[:, t, ki2]))
                    ops.append((lhs_da, whv[:, t, ki2]))
                for n, (l, r) in enumerate(ops):
                    nc.tensor.matmul(
                        ps[:, t * 256 : (t + 1) * 256],
                        l,
                        r,
                        start=(n == 0),
                        stop=(n == len(ops) - 1),
                        perf_mode=DR,
                    )
            if bh_sb_for(toggle) is not None:
                nc.vector.tensor_scalar_mul(
                    out=otile[:, j, :], in0=ps[:], scalar1=1.0 / WH_SCALE
                )
                nc.vector.tensor_add(
                    out=otile[:, j, :], in0=otile[:, j, :], in1=bh_sb_for(toggle)[:]
                )
            elif j % 2 == 0:
                nc.vector.tensor_scalar_mul(
                    out=otile[:, j, :], in0=ps[:], scalar1=1.0 / WH_SCALE
                )
            else:
                nc.scalar.activation(
                    out=otile[:, j, :], in_=ps[:], func=AF.Identity,
                    bias=0.0, scale=1.0 / WH_SCALE,
                )

        _bh_tiles = {}

        def bh_sb_for(key):
            return _bh_tiles.get(key)

        if bh_d is not None:
            bh0 = whpool.tile([P, 512], F32, tag="bh", bufs=2, name="bh0")
            nc.sync.dma_start(out=bh0[:], in_=_bcast_ap(bh_d[0:512]))
            _bh_tiles[0] = bh0
            bh1 = whpool.tile([P, 512], F32, tag="bh", bufs=2, name="bh1")
            nc.sync.dma_start(out=bh1[:], in_=_bcast_ap(bh_d[512:1024]))
            _bh_tiles[1] = bh1

        for t in range(SEQ // 512):
            for n in range(FC):
                ffn1_group(n, t)
                if t == 1 and n % 2 == 1:
                    j = n // 2
                    h2ts[j] = s6_main(j)

        for k in range(NT + 3):
            if 4 <= k < NT:
                h2ts[k] = s6_main(k)
            if 1 <= k <= NT:
                s6_trans(k - 1)
            if 2 <= k <= NT + 1:
                head_j(whv0, dwv0, otile0, k - 2, 0)
            if 3 <= k <= NT + 2:
                head_j(whv1, dwv1, otile1, k - 3, 1)
        out_rr = out_d[:].rearrange("(j p) v -> p j v", p=P)
        nc.sync.dma_start(out=out_rr[:, :, 0:512], in_=otile0[:])
        nc.sync.dma_start(out=out_rr[:, :, 512:1024], in_=otile1[:])

        p_f1.release()
        p_h1.release()

        # ---------- stage 7: vocab head (vc >= 2) ----------
        out_r = out_d[:].rearrange("(j p) v -> p j v", p=P)
        for vc in range(2, NV):
            whv, dwv = load_whv(vc, str(vc))
            if bh_d is not None:
                bh_bc = whpool.tile([P, 512], F32, tag="bh", bufs=2, name=f"bh{vc}")
                nc.sync.dma_start(
                    out=bh_bc[:], in_=_bcast_ap(bh_d[vc * 512 : (vc + 1) * 512])
                )
                _bh_tiles[vc] = bh_bc
            otile = opool.tile([P, NT, 512], BF16 if OUT_BF16 else F32, tag="ot")
            vs = slice(vc * 512, (vc + 1) * 512)
            # split stores per j-half (last chunk: per j-pair) to shrink the
            # final DMA drain after the last matmul
            if vc == NV - 1:
                for j in range(NT):
                    head_j(whv, dwv, otile, j, vc)
                    if j % 2 == 1:
                        nc.sync.dma_start(
                            out=out_r[:, j - 1 : j + 1, vs],
                            in_=otile[:, j - 1 : j + 1, :],
                        )
            else:
                for j in range(NT):
                    head_j(whv, dwv, otile, j, vc)
                    if j == NT // 2 - 1 or j == NT - 1:
                        h0 = j + 1 - NT // 2
                        nc.sync.dma_start(
                            out=out_r[:, h0 : j + 1, vs],
                            in_=otile[:, h0 : j + 1, :],
                        )

        whpool.release()
        p_h2T.release()
        opool.release()
        psum_t.release()
        psum.release()
        small.release()
        const.release()

    nc.finalize()
    return nc


_PROGRAM_CACHE: dict = {}


def _get_program(flags: dict) -> bass.Bass:
    key = tuple(sorted(flags.items()))
    if key not in _PROGRAM_CACHE:
        _PROGRAM_CACHE[key] = _build_program(flags)
    return _PROGRAM_CACHE[key]


def _prep(x, embed_tab, row_embed, col_embed, Wq, bq, Wk, bk, Wv, bv, Wo, bo,
          ln1_g, ln1_b, W1, b1, W2, b2, ln2_g, ln2_b, Wh, bh):
    """Shared host-side prep: flags, common input map, per-core x shards."""
    f32c = lambda a: np.ascontiguousarray(np.asarray(a, dtype=np.float32))
    x = np.asarray(x)
    B = x.shape[0]
    assert x.shape == (B, SEQ)

    import ml_dtypes

    bf16 = ml_dtypes.bfloat16
    fp8 = ml_dtypes.float8_e4m3
    arrs = dict(wo=f32c(Wo))
    arrs["emb"] = np.ascontiguousarray(f32c(embed_tab).astype(bf16))
    arrs["w1"] = np.ascontiguousarray(f32c(W1).astype(bf16))
    arrs["w2"] = np.ascontiguousarray(f32c(W2).astype(bf16))
    arrs["wq"] = np.ascontiguousarray(f32c(Wq).astype(bf16))
    arrs["wk"] = np.ascontiguousarray(f32c(Wk).astype(bf16))
    arrs["wv"] = np.ascontiguousarray(f32c(Wv).astype(bf16))
    # fp8 split head weights, pre-scaled by WH_SCALE:
    #   wh8 = fp8(Wh*S), dwh8 = fp8(Wh*S - wh8); layout [p, vc, t, ki2, i, n]
    whs = f32c(Wh) * WH_SCALE
    w8 = whs.astype(fp8)
    dw8 = (whs - w8.astype(np.float32)).astype(fp8)

    def _wh_layout(a):
        # [D=512, V] -> [ki2, i, p, vc, t, n] -> [p, vc, t, ki2, i, n]
        a = a.reshape(2, 2, P, NV, 2, 256)
        return np.ascontiguousarray(a.transpose(2, 3, 4, 0, 1, 5))

    arrs["wh8"] = _wh_layout(w8)
    arrs["dwh8"] = _wh_layout(dw8)
    pos = np.concatenate(
        [np.repeat(f32c(row_embed), GW, axis=0), np.tile(f32c(col_embed), (GH, 1))],
        axis=-1,
    )
    arrs["pos"] = np.ascontiguousarray(pos.astype(bf16))
    arrs["maskt"] = np.ascontiguousarray(_mask_tiles().astype(bf16))

    bias_map = dict(
        bq=f32c(bq), bk=f32c(bk), bv=f32c(bv), bo=f32c(bo), b1=f32c(b1),
        b2=f32c(b2), bh=f32c(bh), be1=f32c(ln1_b), be2=f32c(ln2_b),
    )
    gain_map = dict(g1=f32c(ln1_g), g2=f32c(ln2_g))
    flags = {k: bool(np.any(v)) for k, v in bias_map.items()}
    flags.update({k: bool(np.any(v != 1.0)) for k, v in gain_map.items()})
    for k, v in {**bias_map, **gain_map}.items():
        if flags[k]:
            arrs[k] = v

    xs = [np.ascontiguousarray(x[c].astype(np.int32)) for c in range(B)]
    return flags, arrs, xs, B


def kernel(**inputs):
    flags, arrs, xs, B = _prep(**inputs)
    nc = _get_program(flags)
    core_ids = list(range(8))
    in_maps = [{**arrs, "x": xs[c % B]} for c in core_ids]
    res = run_bass_kernel_spmd(nc, in_maps, core_ids)
    out = np.stack([res.results[c]["out"] for c in range(B)], axis=0)
    return np.asarray(out, dtype=np.float32)



# revision 57
# speedup vs baseline: 1.0703x; 1.0152x over previous
"""Trainium2 Bass kernel for a small autoregressive transformer block with
local-windowed causal attention and a large (16k) vocab head.

Data-parallel over batch: batch item b runs on NeuronCore b (8 cores).
Per core:
  h   = embed_tab[x] + pos                      [1024, 512]
  q/k/v = h @ Wq/k/v (+b)                       [1024, 512]
  s   = q @ k^T / sqrt(D) + local_causal_mask   (banded, window <= 298)
  o   = softmax(s) @ v @ Wo (+bo)
  h1  = LN(h + o);  f = relu(h1@W1+b1)@W2+b2;  h2 = LN(h1 + f)
  out = h2 @ Wh (+bh)                           [1024, 16384]

Body matmuls run as float32r (full-rate fp32 with N=512 moving dim). The
vocab head runs as fp8e4m3 DoubleRow (256-deep contraction at 0.5 cyc/row)
with a 3-pass residual split (a@w + a@dw + da@w, Wh pre-scaled by 64) to
stay within the 2e-2 error budget.
kernel(**inputs) takes full unsharded inputs, returns [8, 1024, 16384] f32.
"""

import math
import numpy as np

import concourse.bass as bass
import concourse.mybir as mybir
import concourse.tile as tile
from concourse import bacc
from concourse.bass_utils import run_bass_kernel_spmd
from concourse.masks import make_identity

# ---- problem constants (hardcoded per contract) ----
GH = 32
GW = 32
SEQ = 1024
WIN = 9
D = 512
DFF = 1024
VOCAB = 16384
EPS = 1e-5
NEG = -1e30

P = 128
NT = SEQ // P        # 8 token chunks
DC = D // P          # 4 d chunks
FC = DFF // P        # 8 dff chunks
NV = VOCAB // 512    # 32 vocab chunks
INV_SQRT_D = 1.0 / math.sqrt(D)

F32 = mybir.dt.float32
F32R = mybir.dt.float32r
BF16 = mybir.dt.bfloat16
FP8 = mybir.dt.float8e4
I32 = mybir.dt.int32
OUT_BF16 = True
WH_SCALE = 64.0  # Wh pre-scaled by 64 on host; folded out at PSUM eviction
DR = mybir.MatmulPerfMode.DoubleRow
AF = mybir.ActivationFunctionType


def _window_start(i: int) -> int:
    # k-window [ws, ws+512) covers all allowed keys for query chunk i
    # (max lookback is WIN*GW + WIN = 297 < 384).
    return 128 * max(0, i - 3)


def _mask_tiles() -> np.ndarray:
    idx = np.arange(SEQ)
    r, c = idx // GW, idx % GW
    allow = (
        (np.abs(r[:, None] - r[None, :]) <= WIN)
        & (np.abs(c[:, None] - c[None, :]) <= WIN)
        & (idx[None, :] <= idx[:, None])
    )
    maskf = np.where(allow, 0.0, NEG).astype(np.float32)
    tiles = np.empty((NT, P, 512), np.float32)
    for i in range(NT):
        ws = _window_start(i)
        tiles[i] = maskf[i * P : (i + 1) * P, ws : ws + 512]
    return tiles


def _r(ap):
    """bitcast to float32r for full-rate fp32 matmul."""
    return ap.bitcast(F32R)


def _bcast_ap(a: bass.AP) -> bass.AP:
    """[n] DRAM vector AP -> [P, n] partition-broadcast DMA source."""
    return bass.AP(tensor=a.tensor, offset=a.offset, ap=[[0, P], *a.ap])


def _build_program(flags: dict, wh_bufs: int = 6, msk_bufs: int = 6, lean: bool = False) -> bass.Bass:
    nc = bacc.Bacc("TRN2", target_bir_lowering=False)

    # ---------- I/O ----------
    x_d = nc.declare_dram_parameter("x", [SEQ], I32, False)
    emb_d = nc.declare_dram_parameter("emb", [VOCAB, D], BF16, False)
    pos_d = nc.declare_dram_parameter("pos", [SEQ, D], BF16, False)
    msk_d = nc.declare_dram_parameter("maskt", [NT, P, 512], BF16, False)
    wq_d = nc.declare_dram_parameter("wq", [D, D], BF16, False)
    wk_d = nc.declare_dram_parameter("wk", [D, D], BF16, False)
    wv_d = nc.declare_dram_parameter("wv", [D, D], BF16, False)
    wo_d = nc.declare_dram_parameter("wo", [D, D], F32, False)
    w1_d = nc.declare_dram_parameter("w1", [D, DFF], BF16, False)
    w2_d = nc.declare_dram_parameter("w2", [DFF, D], BF16, False)
    # fp8 head weights: [p, vc, vhalf, ki2, i, 256] with d = 128*(2*ki2+i)+p
    wh8_d = nc.declare_dram_parameter("wh8", [P, NV, 2, 2, 2, 256], FP8, False)
    dwh8_d = nc.declare_dram_parameter("dwh8", [P, NV, 2, 2, 2, 256], FP8, False)
    dp = lambda name, shape: nc.declare_dram_parameter(name, shape, F32, False)
    bq_d = dp("bq", [D]) if flags["bq"] else None
    bk_d = dp("bk", [D]) if flags["bk"] else None
    bv_d = dp("bv", [D]) if flags["bv"] else None
    bo_d = dp("bo", [D]) if flags["bo"] else None
    b1_d = dp("b1", [DFF]) if flags["b1"] else None
    b2_d = dp("b2", [D]) if flags["b2"] else None
    bh_d = dp("bh", [VOCAB]) if flags["bh"] else None
    g1_d = dp("g1", [D]) if flags["g1"] else None
    be1_d = dp("be1", [D]) if flags["be1"] else None
    g2_d = dp("g2", [D]) if flags["g2"] else None
    be2_d = dp("be2", [D]) if flags["be2"] else None
    out_d = nc.declare_dram_parameter("out", [SEQ, VOCAB], BF16 if OUT_BF16 else F32, True)

    with tile.TileContext(nc) as tc:
        # ----- whole-kernel pools -----
        const = tc.alloc_tile_pool(name="const", bufs=1)
        small = tc.alloc_tile_pool(name="small", bufs=8)
        psum = tc.alloc_tile_pool(name="psA", bufs=5, space="PSUM")
        psum_t = tc.alloc_tile_pool(name="psT", bufs=2, space="PSUM")
        opool = tc.alloc_tile_pool(name="outev", bufs=3, side="right")
        p_h2T = tc.alloc_tile_pool(name="h2Tp", bufs=1, side="right")

        ident_f = const.tile([P, P], F32, tag="ident_f")
        ident = const.tile([P, P], F32R, tag="ident")
        ident_b = const.tile([P, P], BF16, tag="ident_b")
        eps_t = const.tile([P, 1], F32, tag="eps")
        nc.vector.memset(eps_t[:], EPS)
        x_sb = const.tile([P, NT], I32, tag="x_sb")
        nc.sync.dma_start(out=x_sb[:], in_=x_d[:].rearrange("(j p) -> p j", p=P))

        def load_col_bias(handle, nchunks, tag):
            # [nchunks*P] DRAM -> [P, nchunks] (chunk m in column m)
            t = const.tile([P, nchunks], F32, tag=tag)
            nc.sync.dma_start(out=t[:], in_=handle[:].rearrange("(m p) -> p m", p=P))
            return t

        def load_bcast(handle, n, tag):
            t = const.tile([P, n], F32, tag=tag)
            nc.sync.dma_start(out=t[:], in_=_bcast_ap(handle[:]))
            return t

        bq_sb = load_col_bias(bq_d, DC, "bq") if bq_d else None
        bk_sb = load_col_bias(bk_d, DC, "bk") if bk_d else None
        b1_sb = load_col_bias(b1_d, FC, "b1") if b1_d else None
        bv_bc = load_bcast(bv_d, D, "bv") if bv_d else None
        bo_bc = load_bcast(bo_d, D, "bo") if bo_d else None
        b2_bc = load_bcast(b2_d, D, "b2") if b2_d else None
        g1_bc = load_bcast(g1_d, D, "g1") if g1_d else None
        be1_bc = load_bcast(be1_d, D, "be1") if be1_d else None
        g2_bc = load_bcast(g2_d, D, "g2") if g2_d else None
        be2_bc = load_bcast(be2_d, D, "be2") if be2_d else None

        a8T = [p_h2T.tile([P, DC, P], FP8, tag=f"a8T{j}", name=f"a8T{j}") for j in range(NT)]
        da8T = [p_h2T.tile([P, DC, P], FP8, tag=f"da8T{j}", name=f"da8T{j}") for j in range(NT)]

        # ----- phase A pools (left, LIFO) -----
        p_woh = tc.alloc_tile_pool(name="woh", bufs=1)         # wo, h  (-> stage 4)
        wo_sb = p_woh.tile([P, DC, D], F32R, tag="wo")
        h_sb = p_woh.tile([P, NT, D], F32R, tag="h")

        p_oT = tc.alloc_tile_pool(name="oTp", bufs=1)          # oT    (-> stage 4)
        oT = p_oT.tile([P, DC, SEQ], F32R, tag="oT")

        p_v = tc.alloc_tile_pool(name="vp", bufs=1)            # v (-> wave 2)
        v_sb = p_v.tile([P, NT, D], BF16, tag="v")
        p_at = tc.alloc_tile_pool(name="attnw", bufs=3)        # softmax work (-> stage 4)
        p_qk = tc.alloc_tile_pool(name="qkp", bufs=1)          # qT,kT (-> wave 1)
        qT = p_qk.tile([P, DC, SEQ], F32R, tag="qT")
        kT = p_qk.tile([P, DC, SEQ], F32R, tag="kT")

        p_wq = tc.alloc_tile_pool(name="wqp", bufs=1)          # wq,wk,wv,hT (-> stage 2)
        wq_sb = p_wq.tile([P, DC, D], BF16, tag="wq")
        wk_sb = p_wq.tile([P, DC, D], BF16, tag="wk")
        wv_sb = p_wq.tile([P, DC, D], BF16, tag="wv")
        hT = p_wq.tile([P, DC, SEQ], BF16, tag="hT")

        # ---------- stage 1: embedding gather + positional + transpose ----------
        # interleave gather_j / pos_j DMA issue so chunk j's inputs land together
        # NOTE: multi-offset indirect DMA (several offsets per partition)
        # returns wrong data on real hw — keep one gather per 128-token chunk.
        hb_ts = []
        pos_ts = []
        for jj in range(NT):
            hb_t = p_wq.tile([P, D], BF16, tag="hb", bufs=NT, name=f"hb{jj}")
            nc.gpsimd.indirect_dma_start(
                out=hb_t[:],
                out_offset=None,
                in_=emb_d[:],
                in_offset=bass.IndirectOffsetOnAxis(ap=x_sb[:, jj : jj + 1], axis=0),
            )
            hb_ts.append(hb_t)
            pos_t = p_wq.tile([P, D], BF16, tag="pos", bufs=NT, name=f"pos{jj}")
            nc.sync.dma_start(out=pos_t[:], in_=pos_d[jj * P : (jj + 1) * P, :])
            pos_ts.append(pos_t)
            if jj == 3:
                nc.sync.dma_start(
                    out=wq_sb[:], in_=wq_d[:].rearrange("(k p) o -> p k o", p=P)
                )
            elif jj == 5:
                nc.sync.dma_start(
                    out=wk_sb[:], in_=wk_d[:].rearrange("(k p) o -> p k o", p=P)
                )
            elif jj == 7:
                nc.sync.dma_start(
                    out=wv_sb[:], in_=wv_d[:].rearrange("(k p) o -> p k o", p=P)
                )

        make_identity(nc, ident_f[:])
        nc.vector.tensor_copy(out=ident[:], in_=ident_f[:])
        nc.gpsimd.tensor_copy(out=ident_b[:], in_=ident_f[:])

        def s1_add(j):
            nc.vector.tensor_add(out=h_sb[:, j, :], in0=hb_ts[j][:], in1=pos_ts[j][:])

        def s1_trans(j):
            pt = psum_t.tile([P, 512], F32, tag="pt", name=f"s1pt{j}")
            for m in range(DC):
                nc.tensor.transpose(
                    out=_r(pt[:, m * P : (m + 1) * P]),
                    in_=_r(h_sb[:, j, m * P : (m + 1) * P]),
                    identity=_r(ident[:]),
                )
            nc.scalar.copy(out=hT[:, :, j * P : (j + 1) * P], in_=pt[:])

        # ---------- stage 2: qT / kT (d-major), v (token-major) ----------
        # t-major order: all groups needing hT[0:512] first (PE is in-order)
        def s2_qk(wt, bt, dst, t):
            for m in range(DC):
                ps = psum.tile([P, 512], F32, tag="ps")
                for ki in range(DC):
                    nc.tensor.matmul(
                        ps[:],
                        wt[:, ki, m * P : (m + 1) * P],
                        hT[:, ki, t * 512 : (t + 1) * 512],
                        start=(ki == 0),
                        stop=(ki == DC - 1),
                    )
                dslc = dst[:, m, t * 512 : (t + 1) * 512]
                if bt is not None:
                    nc.scalar.activation(
                        out=dslc, in_=ps[:], func=AF.Identity,
                        bias=bt[:, m : m + 1], scale=1.0,
                    )
                elif dst is kT:
                    nc.vector.tensor_copy(out=dslc, in_=ps[:])
                else:
                    nc.scalar.copy(out=dslc, in_=ps[:])

        def s2_v1(j):
            ps = psum.tile([P, 512], F32, tag="ps")
            for ki in range(DC):
                nc.tensor.matmul(
                    ps[:],
                    hT[:, ki, j * P : (j + 1) * P],
                    wv_sb[:, ki, :],
                    start=(ki == 0),
                    stop=(ki == DC - 1),
                )
            if bv_bc is not None:
                nc.scalar.activation(
                    out=v_sb[:, j, :], in_=ps[:], func=AF.Identity,
                    bias=0.0, scale=1.0,
                )
                nc.vector.tensor_add(out=v_sb[:, j, :], in0=v_sb[:, j, :], in1=bv_bc[:])
            else:
                nc.scalar.copy(out=v_sb[:, j, :], in_=ps[:])

        # ---------- stage 3 wave 1 helper: scores + softmax for chunk i ----------
        attns = [None] * NT
        recips = [None] * NT

        def s3_scores(i):
            ws = _window_start(i)
            nw = min(512, max(256, (i + 1) * P))  # live window (>=256 keeps f32r fast)
            ps_s = psum.tile([P, 512], F32, tag="ps")
            for ki in range(DC):
                nc.tensor.matmul(
                    ps_s[:, :nw],
                    _r(qT[:, ki, i * P : (i + 1) * P]),
                    _r(kT[:, ki, ws : ws + nw]),
                    start=(ki == 0),
                    stop=(ki == DC - 1),
                )
            msk_t = p_at.tile([P, 512], BF16, tag="msk", bufs=4)
            nc.sync.dma_start(out=msk_t[:], in_=msk_d[i])
            s_t = p_at.tile([P, 512], F32, tag="s_t", bufs=2)
            nc.vector.tensor_add(out=s_t[:, :nw], in0=ps_s[:, :nw], in1=msk_t[:, :nw])
            attn = p_at.tile([P, 512], BF16, tag="attn", bufs=NT, name=f"attn{i}")
            denom = small.tile([P, 1], F32, tag="denom")
            nc.scalar.activation(
                out=attn[:, :nw], in_=s_t[:, :nw], func=AF.Exp,
                bias=0.0, scale=INV_SQRT_D,
                accum_out=denom[:, 0:1],
            )
            recip = small.tile([P, 1], F32, tag="recip", bufs=NT, name=f"recip{i}")
            nc.vector.reciprocal(out=recip[:], in_=denom[:])
            attns[i] = attn
            recips[i] = recip

        # chunk-granular pipeline: v(j) fills PE while gathers trickle in;
        # scores 0-3 run as soon as t=0 qT/kT exist
        for k in range(5):
            if k < 4:
                s1_add(k)
            if k >= 1:
                s1_trans(k - 1)
        # adds for chunks 4-7 issued now so DVE never blocks them behind
        # stage-2 eviction work; their transposes run after stage-2 t=0
        for k in range(4, NT):
            s1_add(k)

        # wo load after the latency-critical stage-1 stream
        nc.sync.dma_start(
            out=wo_sb[:], in_=_r(wo_d[:].rearrange("(k p) o -> p k o", p=P))
        )

        s2_qk(wq_sb, bq_sb, qT, 0)
        s2_qk(wk_sb, bk_sb, kT, 0)
        for j in range(4):
            s2_v1(j)
        for i in range(4):
            s3_scores(i)
        for k in range(4, NT):
            s1_trans(k)
            s2_v1(k)
        s2_qk(wq_sb, bq_sb, qT, 1)
        s2_qk(wk_sb, bk_sb, kT, 1)
        for i in range(4, NT):
            s3_scores(i)

        p_wq.release()
        p_qk.release()

        # ----- right-side pools for FFN phase -----
        whpool = tc.alloc_tile_pool(name="whstream", bufs=8, side="right")
        p_h1 = tc.alloc_tile_pool(name="h1p", bufs=1, side="right")
        h1_sb = p_h1.tile([P, NT, D], F32R, tag="h1")
        h1T = p_h1.tile([P, DC, SEQ], BF16, tag="h1T")
        w1_sb = p_h1.tile([P, DC, DFF], BF16, tag="w1")
        nc.sync.dma_start(out=w1_sb[:], in_=w1_d[:].rearrange("(k p) o -> p k o", p=P))
        w2_sb = p_h1.tile([P, FC, D], BF16, tag="w2")
        nc.sync.dma_start(out=w2_sb[:], in_=w2_d[:].rearrange("(k p) o -> p k o", p=P))

        # ---------- stage 3 wave 2 + stage 4, software-pipelined ----------
        p_st4 = tc.alloc_tile_pool(name="st4", bufs=3)
        attnTs = [None] * NT
        o_ts = [None] * NT

        def w2_a(i):  # attn transposes + attnT eviction
            ws = _window_start(i)
            kb0 = ws // P
            nkb = min(DC, i - kb0 + 1)
            pt = psum_t.tile([P, 512], BF16, tag="ptb", bufs=1, name=f"atp{i}")
            for kk in range(nkb):
                nc.tensor.transpose(
                    out=pt[:, kk * P : (kk + 1) * P],
                    in_=attns[i][:, kk * P : (kk + 1) * P],
                    identity=ident_b[:],
                )
            attnT = p_at.tile([P, 512], BF16, tag="attnT", bufs=3, name=f"attnT{i}")
            nc.vector.tensor_copy(out=attnT[:, : nkb * P], in_=pt[:, : nkb * P])
            attnTs[i] = attnT

        def w2_b(i):  # o matmuls + scale
            ws = _window_start(i)
            kb0 = ws // P
            nkb = min(DC, i - kb0 + 1)
            ps_o = psum.tile([P, 512], F32, tag="ps", name=f"pso{i}")
            for kk in range(nkb):
                nc.tensor.matmul(
                    ps_o[:],
                    attnTs[i][:, kk * P : (kk + 1) * P],
                    v_sb[:, kb0 + kk, :],
                    start=(kk == 0),
                    stop=(kk == nkb - 1),
                )
            o_t = p_at.tile([P, D], F32R, tag="o_t", bufs=3, name=f"o_t{i}")
            nc.vector.tensor_scalar_mul(out=o_t[:], in0=ps_o[:], scalar1=recips[i][:, 0:1])
            o_ts[i] = o_t

        def w2_c(i):  # oT transposes + eviction
            pt2 = psum_t.tile([P, 512], F32, tag="pt", name=f"otp{i}")
            for m in range(DC):
                nc.tensor.transpose(
                    out=_r(pt2[:, m * P : (m + 1) * P]),
                    in_=_r(o_ts[i][:, m * P : (m + 1) * P]),
                    identity=_r(ident[:]),
                )
            nc.vector.tensor_copy(out=oT[:, :, i * P : (i + 1) * P], in_=pt2[:])

        def s4_proj(j):  # attn projection + residual + LN1 (no transpose)
            ps = psum.tile([P, 512], F32, tag="ps", name=f"psp{j}")
            for m in range(DC):
                nc.tensor.matmul(
                    ps[:],
                    _r(oT[:, m, j * P : (j + 1) * P]),
                    _r(wo_sb[:, m, :]),
                    start=(m == 0),
                    stop=(m == DC - 1),
                )
            r1 = p_st4.tile([P, D], F32, tag="r1", name=f"r1_{j}")
            nc.vector.tensor_add(out=r1[:], in0=h_sb[:, j, :], in1=ps[:])
            if bo_bc is not None:
                nc.vector.tensor_add(out=r1[:], in0=r1[:], in1=bo_bc[:])
            stats = small.tile([P, 6], F32, tag="stats")
            nc.vector.bn_stats(out=stats[:], in_=r1[:])
            mv = small.tile([P, 2], F32, tag="mv")
            nc.vector.bn_aggr(out=mv[:], in_=stats[:])
            stdt = small.tile([P, 1], F32, tag="stdt")
            nc.scalar.activation(
                out=stdt[:], in_=mv[:, 1:2], func=AF.Sqrt,
                bias=eps_t[:, 0:1], scale=1.0,
            )
            rstd = small.tile([P, 1], F32, tag="rstd")
            nc.vector.reciprocal(out=rstd[:], in_=stdt[:])
            nc.vector.tensor_scalar(
                out=h1_sb[:, j, :], in0=r1[:],
                scalar1=mv[:, 0:1], scalar2=rstd[:, 0:1],
                op0=mybir.AluOpType.subtract, op1=mybir.AluOpType.mult,
            )
            if g1_bc is not None:
                nc.vector.tensor_mul(out=h1_sb[:, j, :], in0=h1_sb[:, j, :], in1=g1_bc[:])
            if be1_bc is not None:
                nc.vector.tensor_add(out=h1_sb[:, j, :], in0=h1_sb[:, j, :], in1=be1_bc[:])

        def s4_trans(j):  # h1 transposes + h1T eviction
            pt3 = psum_t.tile([P, 512], F32, tag="pt", name=f"h1p{j}")
            for m in range(DC):
                nc.tensor.transpose(
                    out=_r(pt3[:, m * P : (m + 1) * P]),
                    in_=_r(h1_sb[:, j, m * P : (m + 1) * P]),
                    identity=_r(ident[:]),
                )
            nc.scalar.copy(out=h1T[:, :, j * P : (j + 1) * P], in_=pt3[:])

        for k in range(NT + 6):
            if k < NT:
                w2_a(k)
            if 1 <= k < NT + 1:
                w2_b(k - 1)
            if 3 <= k < NT + 3:
                w2_c(k - 3)
            if 4 <= k < NT + 4:
                s4_proj(k - 4)
            if 6 <= k:
                s4_trans(k - 6)

        p_st4.release()
        p_at.release()
        p_v.release()
        p_oT.release()
        p_woh.release()

        # ---------- stage 5: FFN up, f1T = relu(W1^T @ h1T + b1) ----------
        p_f1 = tc.alloc_tile_pool(name="f1p", bufs=1, side="right")
        f1T = p_f1.tile([P, FC, SEQ], BF16, tag="f1T")
        def ffn1_group(n, t):
            ps = psum.tile([P, 512], F32, tag="ps", name=f"psf{n}_{t}")
            for ki in range(DC):
                nc.tensor.matmul(
                    ps[:],
                    w1_sb[:, ki, n * P : (n + 1) * P],
                    h1T[:, ki, t * 512 : (t + 1) * 512],
                    start=(ki == 0),
                    stop=(ki == DC - 1),
                )
            fslc = f1T[:, n, t * 512 : (t + 1) * 512]
            if b1_sb is not None:
                nc.vector.tensor_scalar(
                    out=fslc, in0=ps[:],
                    scalar1=b1_sb[:, n : n + 1], scalar2=0.0,
                    op0=mybir.AluOpType.add, op1=mybir.AluOpType.max,
                )
            else:
                nc.vector.tensor_scalar_max(out=fslc, in0=ps[:], scalar1=0.0)

        # ---------- stage 6: FFN down + residual + LN2 (pipelined) ----------
        def s6_main(j):
            ps = psum.tile([P, 512], F32, tag="ps", name=f"ps6_{j}")
            for n in range(FC):
                nc.tensor.matmul(
                    ps[:],
                    f1T[:, n, j * P : (j + 1) * P],
                    w2_sb[:, n, :],
                    start=(n == 0),
                    stop=(n == FC - 1),
                )
            r2 = p_f1.tile([P, D], F32, tag="r2", bufs=3, name=f"r2_{j}")
            nc.vector.tensor_add(out=r2[:], in0=h1_sb[:, j, :], in1=ps[:])
            if b2_bc is not None:
                nc.vector.tensor_add(out=r2[:], in0=r2[:], in1=b2_bc[:])
            stats = small.tile([P, 6], F32, tag="stats")
            nc.vector.bn_stats(out=stats[:], in_=r2[:])
            mv = small.tile([P, 2], F32, tag="mv")
            nc.vector.bn_aggr(out=mv[:], in_=stats[:])
            stdt = small.tile([P, 1], F32, tag="stdt")
            nc.scalar.activation(
                out=stdt[:], in_=mv[:, 1:2], func=AF.Sqrt,
                bias=eps_t[:, 0:1], scale=1.0,
            )
            rstd = small.tile([P, 1], F32, tag="rstd")
            nc.vector.reciprocal(out=rstd[:], in_=stdt[:])
            h2_t = p_f1.tile([P, D], F32R, tag="h2_t", bufs=3, name=f"h2t_{j}")
            nc.vector.tensor_scalar(
                out=h2_t[:], in0=r2[:],
                scalar1=mv[:, 0:1], scalar2=rstd[:, 0:1],
                op0=mybir.AluOpType.subtract, op1=mybir.AluOpType.mult,
            )
            if g2_bc is not None:
                nc.vector.tensor_mul(out=h2_t[:], in0=h2_t[:], in1=g2_bc[:])
            if be2_bc is not None:
                nc.vector.tensor_add(out=h2_t[:], in0=h2_t[:], in1=be2_bc[:])
            return h2_t

        h2ts = [None] * NT

        def s6_trans(j):
            # transpose h2, then split-quantize to fp8: a8T = fp8(h2T),
            # da8T = fp8(h2T - a8T)
            pt = psum_t.tile([P, DC, P], F32, tag="pt", name=f"h2p{j}")
            for m in range(DC):
                nc.tensor.transpose(
                    out=_r(pt[:, m, :]),
                    in_=_r(h2ts[j][:, m * P : (m + 1) * P]),
                    identity=_r(ident[:]),
                )
            nc.scalar.copy(out=a8T[j][:, :, :], in_=pt[:, :, :])
            da_t = p_f1.tile([P, DC, P], F32, tag="da_t", bufs=2, name=f"da_t{j}")
            nc.vector.tensor_sub(out=da_t[:, :, :], in0=pt[:, :, :], in1=a8T[j][:, :, :])
            nc.gpsimd.tensor_copy(out=da8T[j][:, :, :], in_=da_t[:, :, :])

        # head chunks for vc=0,1 interleaved into stage-6 so PE fills LN waits
        def load_whv(vc, nm):
            whv = whpool.tile([P, 2, 2, 2, 256], FP8, tag="whv", name=f"whv{nm}")
            nc.sync.dma_start(out=whv[:], in_=wh8_d[:, vc])
            dwv = whpool.tile([P, 2, 2, 2, 256], FP8, tag="dwv", name=f"dwv{nm}")
            nc.sync.dma_start(out=dwv[:], in_=dwh8_d[:, vc])
            return whv, dwv

        whv0, dwv0 = load_whv(0, "0")
        otile0 = opool.tile([P, NT, 512], BF16 if OUT_BF16 else F32, tag="ot", name="otile0")
        whv1, dwv1 = load_whv(1, "1")
        otile1 = opool.tile([P, NT, 512], BF16 if OUT_BF16 else F32, tag="ot", name="otile1")

        def head_j(whv, dwv, otile, j, toggle):
            # 3-pass fp8 DoubleRow: a@w + a@dw + da@w, one PSUM group per
            # 256-vocab half; scale 1/WH_SCALE folded into the eviction
            ps = psum.tile([P, 512], F32, tag="ps", name=f"psh{toggle}_{j}")
            for t in range(2):
                ops = []
                for ki2 in range(2):
                    lhs_a = a8T[j][:, 2 * ki2 : 2 * ki2 + 2, :]
                    lhs_da = da8T[j][:, 2 * ki2 : 2 * ki2 + 2, :]
                    ops.append((lhs_a, whv[:, t, ki2]))
                    ops.append((lhs_a, dwv[:, t, ki2]))
                    ops.append((lhs_da, whv[:, t, ki2]))
                for n, (l, r) in enumerate(ops):
                    nc.tensor.matmul(
                        ps[:, t * 256 : (t + 1) * 256],
                        l,
                        r,
                        start=(n == 0),
                        stop=(n == len(ops) - 1),
                        perf_mode=DR,
                    )
            if bh_sb_for(toggle) is not None:
                nc.vector.tensor_scalar_mul(
                    out=otile[:, j, :], in0=ps[:], scalar1=1.0 / WH_SCALE
                )
                nc.vector.tensor_add(
                    out=otile[:, j, :], in0=otile[:, j, :], in1=bh_sb_for(toggle)[:]
                )
            elif j % 2 == 0:
                nc.vector.tensor_scalar_mul(
                    out=otile[:, j, :], in0=ps[:], scalar1=1.0 / WH_SCALE
                )
            else:
                nc.scalar.activation(
                    out=otile[:, j, :], in_=ps[:], func=AF.Identity,
                    bias=0.0, scale=1.0 / WH_SCALE,
                )

        _bh_tiles = {}

        def bh_sb_for(key):
            return _bh_tiles.get(key)

        if bh_d is not None:
            bh0 = whpool.tile([P, 512], F32, tag="bh", bufs=2, name="bh0")
            nc.sync.dma_start(out=bh0[:], in_=_bcast_ap(bh_d[0:512]))
            _bh_tiles[0] = bh0
            bh1 = whpool.tile([P, 512], F32, tag="bh", bufs=2, name="bh1")
            nc.sync.dma_start(out=bh1[:], in_=_bcast_ap(bh_d[512:1024]))
            _bh_tiles[1] = bh1

        for t in range(SEQ // 512):
            for n in range(FC):
                ffn1_group(n, t)
                if t == 1 and n % 2 == 1:
                    j = n // 2
                    h2ts[j] = s6_main(j)

        for k in range(NT + 5):
            if 4 <= k < NT:
                h2ts[k] = s6_main(k)
            if 2 <= k <= NT + 1:
                s6_trans(k - 2)
            if 4 <= k <= NT + 3:
                head_j(whv0, dwv0, otile0, k - 4, 0)
            if 5 <= k <= NT + 4:
                head_j(whv1, dwv1, otile1, k - 5, 1)
        out_rr = out_d[:].rearrange("(j p) v -> p j v", p=P)
        nc.sync.dma_start(out=out_rr[:, :, 0:512], in_=otile0[:])
        nc.sync.dma_start(out=out_rr[:, :, 512:1024], in_=otile1[:])

        p_f1.release()
        p_h1.release()

        # ---------- stage 7: vocab head (vc >= 2) ----------
        out_r = out_d[:].rearrange("(j p) v -> p j v", p=P)
        for vc in range(2, NV):
            whv, dwv = load_whv(vc, str(vc))
            if bh_d is not None:
                bh_bc = whpool.tile([P, 512], F32, tag="bh", bufs=2, name=f"bh{vc}")
                nc.sync.dma_start(
                    out=bh_bc[:], in_=_bcast_ap(bh_d[vc * 512 : (vc + 1) * 512])
                )
                _bh_tiles[vc] = bh_bc
            otile = opool.tile([P, NT, 512], BF16 if OUT_BF16 else F32, tag="ot")
            vs = slice(vc * 512, (vc + 1) * 512)
            # split stores per j-half (last chunk: per j-pair) to shrink the
            # final DMA drain after the last matmul
            if vc == NV - 1:
                for j in range(NT):
                    head_j(whv, dwv, otile, j, vc)
                    if j % 2 == 1:
                        nc.sync.dma_start(
                            out=out_r[:, j - 1 : j + 1, vs],
                            in_=otile[:, j - 1 : j + 1, :],
                        )
            else:
                for j in range(NT):
                    head_j(whv, dwv, otile, j, vc)
                    if j == NT // 2 - 1 or j == NT - 1:
                        h0 = j + 1 - NT // 2
                        nc.sync.dma_start(
                            out=out_r[:, h0 : j + 1, vs],
                            in_=otile[:, h0 : j + 1, :],
                        )

        whpool.release()
        p_h2T.release()
        opool.release()
        psum_t.release()
        psum.release()
        small.release()
        const.release()

    nc.finalize()
    return nc


_PROGRAM_CACHE: dict = {}


def _get_program(flags: dict) -> bass.Bass:
    key = tuple(sorted(flags.items()))
    if key not in _PROGRAM_CACHE:
        _PROGRAM_CACHE[key] = _build_program(flags)
    return _PROGRAM_CACHE[key]


def _prep(x, embed_tab, row_embed, col_embed, Wq, bq, Wk, bk, Wv, bv, Wo, bo,
          ln1_g, ln1_b, W1, b1, W2, b2, ln2_g, ln2_b, Wh, bh):
    """Shared host-side prep: flags, common input map, per-core x shards."""
    f32c = lambda a: np.ascontiguousarray(np.asarray(a, dtype=np.float32))
    x = np.asarray(x)
    B = x.shape[0]
    assert x.shape == (B, SEQ)

    import ml_dtypes

    bf16 = ml_dtypes.bfloat16
    fp8 = ml_dtypes.float8_e4m3
    arrs = dict(wo=f32c(Wo))
    arrs["emb"] = np.ascontiguousarray(f32c(embed_tab).astype(bf16))
    arrs["w1"] = np.ascontiguousarray(f32c(W1).astype(bf16))
    arrs["w2"] = np.ascontiguousarray(f32c(W2).astype(bf16))
    arrs["wq"] = np.ascontiguousarray(f32c(Wq).astype(bf16))
    arrs["wk"] = np.ascontiguousarray(f32c(Wk).astype(bf16))
    arrs["wv"] = np.ascontiguousarray(f32c(Wv).astype(bf16))
    # fp8 split head weights, pre-scaled by WH_SCALE:
    #   wh8 = fp8(Wh*S), dwh8 = fp8(Wh*S - wh8); layout [p, vc, t, ki2, i, n]
    whs = f32c(Wh) * WH_SCALE
    w8 = whs.astype(fp8)
    dw8 = (whs - w8.astype(np.float32)).astype(fp8)

    def _wh_layout(a):
        # [D=512, V] -> [ki2, i, p, vc, t, n] -> [p, vc, t, ki2, i, n]
        a = a.reshape(2, 2, P, NV, 2, 256)
        return np.ascontiguousarray(a.transpose(2, 3, 4, 0, 1, 5))

    arrs["wh8"] = _wh_layout(w8)
    arrs["dwh8"] = _wh_layout(dw8)
    pos = np.concatenate(
        [np.repeat(f32c(row_embed), GW, axis=0), np.tile(f32c(col_embed), (GH, 1))],
        axis=-1,
    )
    arrs["pos"] = np.ascontiguousarray(pos.astype(bf16))
    arrs["maskt"] = np.ascontiguousarray(_mask_tiles().astype(bf16))

    bias_map = dict(
        bq=f32c(bq), bk=f32c(bk), bv=f32c(bv), bo=f32c(bo), b1=f32c(b1),
        b2=f32c(b2), bh=f32c(bh), be1=f32c(ln1_b), be2=f32c(ln2_b),
    )
    gain_map = dict(g1=f32c(ln1_g), g2=f32c(ln2_g))
    flags = {k: bool(np.any(v)) for k, v in bias_map.items()}
    flags.update({k: bool(np.any(v != 1.0)) for k, v in gain_map.items()})
    for k, v in {**bias_map, **gain_map}.items():
        if flags[k]:
            arrs[k] = v

    xs = [np.ascontiguousarray(x[c].astype(np.int32)) for c in range(B)]
    return flags, arrs, xs, B


def kernel(**inputs):
    flags, arrs, xs, B = _prep(**inputs)
    nc = _get_program(flags)
    core_ids = list(range(8))
    in_maps = [{**arrs, "x": xs[c % B]} for c in core_ids]
    res = run_bass_kernel_spmd(nc, in_maps, core_ids)
    out = np.stack([res.results[c]["out"] for c in range(B)], axis=0)
    return np.asarray(out, dtype=np.float32)



# revision 59
# speedup vs baseline: 1.0844x; 1.0132x over previous
"""Trainium2 Bass kernel for a small autoregressive transformer block with
local-windowed causal attention and a large (16k) vocab head.

Data-parallel over batch: batch item b runs on NeuronCore b (8 cores).
Per core:
  h   = embed_tab[x] + pos                      [1024, 512]
  q/k/v = h @ Wq/k/v (+b)                       [1024, 512]
  s   = q @ k^T / sqrt(D) + local_causal_mask   (banded, window <= 298)
  o   = softmax(s) @ v @ Wo (+bo)
  h1  = LN(h + o);  f = relu(h1@W1+b1)@W2+b2;  h2 = LN(h1 + f)
  out = h2 @ Wh (+bh)                           [1024, 16384]

Body matmuls run as float32r (full-rate fp32 with N=512 moving dim). The
vocab head runs as fp8e4m3 DoubleRow (256-deep contraction at 0.5 cyc/row)
with a 3-pass residual split (a@w + a@dw + da@w, Wh pre-scaled by 64) to
stay within the 2e-2 error budget.
kernel(**inputs) takes full unsharded inputs, returns [8, 1024, 16384] f32.
"""

import math
import numpy as np

import concourse.bass as bass
import concourse.mybir as mybir
import concourse.tile as tile
from concourse import bacc
from concourse.bass_utils import run_bass_kernel_spmd
from concourse.masks import make_identity

# ---- problem constants (hardcoded per contract) ----
GH = 32
GW = 32
SEQ = 1024
WIN = 9
D = 512
DFF = 1024
VOCAB = 16384
EPS = 1e-5
NEG = -1e30

P = 128
NT = SEQ // P        # 8 token chunks
DC = D // P          # 4 d chunks
FC = DFF // P        # 8 dff chunks
NV = VOCAB // 512    # 32 vocab chunks
INV_SQRT_D = 1.0 / math.sqrt(D)

F32 = mybir.dt.float32
F32R = mybir.dt.float32r
BF16 = mybir.dt.bfloat16
FP8 = mybir.dt.float8e4
I32 = mybir.dt.int32
OUT_BF16 = True
WH_SCALE = 64.0  # Wh pre-scaled by 64 on host; folded out at PSUM eviction
DR = mybir.MatmulPerfMode.DoubleRow
AF = mybir.ActivationFunctionType


def _window_start(i: int) -> int:
    # k-window [ws, ws+512) covers all allowed keys for query chunk i
    # (max lookback is WIN*GW + WIN = 297 < 384).
    return 128 * max(0, i - 3)


def _mask_tiles() -> np.ndarray:
    idx = np.arange(SEQ)
    r, c = idx // GW, idx % GW
    allow = (
        (np.abs(r[:, None] - r[None, :]) <= WIN)
        & (np.abs(c[:, None] - c[None, :]) <= WIN)
        & (idx[None, :] <= idx[:, None])
    )
    maskf = np.where(allow, 0.0, NEG).astype(np.float32)
    tiles = np.empty((NT, P, 512), np.float32)
    for i in range(NT):
        ws = _window_start(i)
        tiles[i] = maskf[i * P : (i + 1) * P, ws : ws + 512]
    return tiles


def _r(ap):
    """bitcast to float32r for full-rate fp32 matmul."""
    return ap.bitcast(F32R)


def _bcast_ap(a: bass.AP) -> bass.AP:
    """[n] DRAM vector AP -> [P, n] partition-broadcast DMA source."""
    return bass.AP(tensor=a.tensor, offset=a.offset, ap=[[0, P], *a.ap])


def _build_program(flags: dict, wh_bufs: int = 6, msk_bufs: int = 6, lean: bool = False) -> bass.Bass:
    nc = bacc.Bacc("TRN2", target_bir_lowering=False)

    # ---------- I/O ----------
    x_d = nc.declare_dram_parameter("x", [SEQ], I32, False)
    emb_d = nc.declare_dram_parameter("emb", [VOCAB, D], BF16, False)
    pos_d = nc.declare_dram_parameter("pos", [SEQ, D], BF16, False)
    msk_d = nc.declare_dram_parameter("maskt", [NT, P, 512], BF16, False)
    wq_d = nc.declare_dram_parameter("wq", [D, D], BF16, False)
    wk_d = nc.declare_dram_parameter("wk", [D, D], BF16, False)
    wv_d = nc.declare_dram_parameter("wv", [D, D], BF16, False)
    wo_d = nc.declare_dram_parameter("wo", [D, D], F32, False)
    w1_d = nc.declare_dram_parameter("w1", [D, DFF], BF16, False)
    w2_d = nc.declare_dram_parameter("w2", [DFF, D], BF16, False)
    # fp8 head weights: [p, vc, vhalf, ki2, i, 256] with d = 128*(2*ki2+i)+p
    wh8_d = nc.declare_dram_parameter("wh8", [P, NV, 2, 2, 2, 256], FP8, False)
    dwh8_d = nc.declare_dram_parameter("dwh8", [P, NV, 2, 2, 2, 256], FP8, False)
    dp = lambda name, shape: nc.declare_dram_parameter(name, shape, F32, False)
    bq_d = dp("bq", [D]) if flags["bq"] else None
    bk_d = dp("bk", [D]) if flags["bk"] else None
    bv_d = dp("bv", [D]) if flags["bv"] else None
    bo_d = dp("bo", [D]) if flags["bo"] else None
    b1_d = dp("b1", [DFF]) if flags["b1"] else None
    b2_d = dp("b2", [D]) if flags["b2"] else None
    bh_d = dp("bh", [VOCAB]) if flags["bh"] else None
    g1_d = dp("g1", [D]) if flags["g1"] else None
    be1_d = dp("be1", [D]) if flags["be1"] else None
    g2_d = dp("g2", [D]) if flags["g2"] else None
    be2_d = dp("be2", [D]) if flags["be2"] else None
    out_d = nc.declare_dram_parameter("out", [SEQ, VOCAB], BF16 if OUT_BF16 else F32, True)

    with tile.TileContext(nc) as tc:
        # ----- whole-kernel pools -----
        const = tc.alloc_tile_pool(name="const", bufs=1)
        small = tc.alloc_tile_pool(name="small", bufs=8)
        psum = tc.alloc_tile_pool(name="psA", bufs=5, space="PSUM")
        psum_t = tc.alloc_tile_pool(name="psT", bufs=2, space="PSUM")
        opool = tc.alloc_tile_pool(name="outev", bufs=3, side="right")
        p_h2T = tc.alloc_tile_pool(name="h2Tp", bufs=1, side="right")

        ident_f = const.tile([P, P], F32, tag="ident_f")
        ident = const.tile([P, P], F32R, tag="ident")
        ident_b = const.tile([P, P], BF16, tag="ident_b")
        eps_t = const.tile([P, 1], F32, tag="eps")
        nc.vector.memset(eps_t[:], EPS)
        x_sb = const.tile([P, NT], I32, tag="x_sb")
        nc.sync.dma_start(out=x_sb[:], in_=x_d[:].rearrange("(j p) -> p j", p=P))

        def load_col_bias(handle, nchunks, tag):
            # [nchunks*P] DRAM -> [P, nchunks] (chunk m in column m)
            t = const.tile([P, nchunks], F32, tag=tag)
            nc.sync.dma_start(out=t[:], in_=handle[:].rearrange("(m p) -> p m", p=P))
            return t

        def load_bcast(handle, n, tag):
            t = const.tile([P, n], F32, tag=tag)
            nc.sync.dma_start(out=t[:], in_=_bcast_ap(handle[:]))
            return t

        bq_sb = load_col_bias(bq_d, DC, "bq") if bq_d else None
        bk_sb = load_col_bias(bk_d, DC, "bk") if bk_d else None
        b1_sb = load_col_bias(b1_d, FC, "b1") if b1_d else None
        bv_bc = load_bcast(bv_d, D, "bv") if bv_d else None
        bo_bc = load_bcast(bo_d, D, "bo") if bo_d else None
        b2_bc = load_bcast(b2_d, D, "b2") if b2_d else None
        g1_bc = load_bcast(g1_d, D, "g1") if g1_d else None
        be1_bc = load_bcast(be1_d, D, "be1") if be1_d else None
        g2_bc = load_bcast(g2_d, D, "g2") if g2_d else None
        be2_bc = load_bcast(be2_d, D, "be2") if be2_d else None

        a8T = [p_h2T.tile([P, DC, P], FP8, tag=f"a8T{j}", name=f"a8T{j}") for j in range(NT)]
        da8T = [p_h2T.tile([P, DC, P], FP8, tag=f"da8T{j}", name=f"da8T{j}") for j in range(NT)]

        # ----- phase A pools (left, LIFO) -----
        p_woh = tc.alloc_tile_pool(name="woh", bufs=1)         # wo, h  (-> stage 4)
        wo_sb = p_woh.tile([P, DC, D], F32R, tag="wo")
        h_sb = p_woh.tile([P, NT, D], F32R, tag="h")

        p_oT = tc.alloc_tile_pool(name="oTp", bufs=1)          # oT    (-> stage 4)
        oT = p_oT.tile([P, DC, SEQ], F32R, tag="oT")

        p_v = tc.alloc_tile_pool(name="vp", bufs=1)            # v (-> wave 2)
        v_sb = p_v.tile([P, NT, D], BF16, tag="v")
        p_at = tc.alloc_tile_pool(name="attnw", bufs=3)        # softmax work (-> stage 4)
        p_qk = tc.alloc_tile_pool(name="qkp", bufs=1)          # qT,kT (-> wave 1)
        qT = p_qk.tile([P, DC, SEQ], F32R, tag="qT")
        kT = p_qk.tile([P, DC, SEQ], F32R, tag="kT")

        p_wq = tc.alloc_tile_pool(name="wqp", bufs=1)          # wq,wk,wv,hT (-> stage 2)
        wq_sb = p_wq.tile([P, DC, D], BF16, tag="wq")
        wk_sb = p_wq.tile([P, DC, D], BF16, tag="wk")
        wv_sb = p_wq.tile([P, DC, D], BF16, tag="wv")
        hT = p_wq.tile([P, DC, SEQ], BF16, tag="hT")

        # ---------- stage 1: embedding gather + positional + transpose ----------
        # interleave gather_j / pos_j DMA issue so chunk j's inputs land together
        # NOTE: multi-offset indirect DMA (several offsets per partition)
        # returns wrong data on real hw — keep one gather per 128-token chunk.
        hb_ts = []
        pos_ts = []
        for jj in range(NT):
            hb_t = p_wq.tile([P, D], BF16, tag="hb", bufs=NT, name=f"hb{jj}")
            nc.gpsimd.indirect_dma_start(
                out=hb_t[:],
                out_offset=None,
                in_=emb_d[:],
                in_offset=bass.IndirectOffsetOnAxis(ap=x_sb[:, jj : jj + 1], axis=0),
            )
            hb_ts.append(hb_t)
            pos_t = p_wq.tile([P, D], BF16, tag="pos", bufs=NT, name=f"pos{jj}")
            nc.sync.dma_start(out=pos_t[:], in_=pos_d[jj * P : (jj + 1) * P, :])
            pos_ts.append(pos_t)
            # fine-grained weight pieces (364ns each) so each gather is
            # displaced by at most one piece in the FIFO DMA queue
            wq_r = wq_d[:].rearrange("(k p) o -> p k o", p=P)
            wk_r = wk_d[:].rearrange("(k p) o -> p k o", p=P)
            if jj >= 2 and jj <= 5:
                ki = jj - 2
                nc.sync.dma_start(out=wq_sb[:, ki, :], in_=wq_r[:, ki, :])
            elif jj >= 6:
                ki = jj - 6
                nc.sync.dma_start(out=wk_sb[:, ki, :], in_=wk_r[:, ki, :])

        make_identity(nc, ident_f[:])
        nc.vector.tensor_copy(out=ident[:], in_=ident_f[:])
        nc.gpsimd.tensor_copy(out=ident_b[:], in_=ident_f[:])

        def s1_add(j):
            nc.vector.tensor_add(out=h_sb[:, j, :], in0=hb_ts[j][:], in1=pos_ts[j][:])

        def s1_trans(j):
            pt = psum_t.tile([P, 512], F32, tag="pt", name=f"s1pt{j}")
            for m in range(DC):
                nc.tensor.transpose(
                    out=_r(pt[:, m * P : (m + 1) * P]),
                    in_=_r(h_sb[:, j, m * P : (m + 1) * P]),
                    identity=_r(ident[:]),
                )
            nc.scalar.copy(out=hT[:, :, j * P : (j + 1) * P], in_=pt[:])

        # ---------- stage 2: qT / kT (d-major), v (token-major) ----------
        # t-major order: all groups needing hT[0:512] first (PE is in-order)
        def s2_qk(wt, bt, dst, t):
            for m in range(DC):
                ps = psum.tile([P, 512], F32, tag="ps")
                for ki in range(DC):
                    nc.tensor.matmul(
                        ps[:],
                        wt[:, ki, m * P : (m + 1) * P],
                        hT[:, ki, t * 512 : (t + 1) * 512],
                        start=(ki == 0),
                        stop=(ki == DC - 1),
                    )
                dslc = dst[:, m, t * 512 : (t + 1) * 512]
                if bt is not None:
                    nc.scalar.activation(
                        out=dslc, in_=ps[:], func=AF.Identity,
                        bias=bt[:, m : m + 1], scale=1.0,
                    )
                elif dst is kT:
                    nc.vector.tensor_copy(out=dslc, in_=ps[:])
                else:
                    nc.scalar.copy(out=dslc, in_=ps[:])

        def s2_v1(j):
            ps = psum.tile([P, 512], F32, tag="ps")
            for ki in range(DC):
                nc.tensor.matmul(
                    ps[:],
                    hT[:, ki, j * P : (j + 1) * P],
                    wv_sb[:, ki, :],
                    start=(ki == 0),
                    stop=(ki == DC - 1),
                )
            if bv_bc is not None:
                nc.scalar.activation(
                    out=v_sb[:, j, :], in_=ps[:], func=AF.Identity,
                    bias=0.0, scale=1.0,
                )
                nc.vector.tensor_add(out=v_sb[:, j, :], in0=v_sb[:, j, :], in1=bv_bc[:])
            else:
                nc.scalar.copy(out=v_sb[:, j, :], in_=ps[:])

        # ---------- stage 3 wave 1 helper: scores + softmax for chunk i ----------
        attns = [None] * NT
        recips = [None] * NT

        def s3_scores(i):
            ws = _window_start(i)
            nw = min(512, max(256, (i + 1) * P))  # live window (>=256 keeps f32r fast)
            ps_s = psum.tile([P, 512], F32, tag="ps")
            for ki in range(DC):
                nc.tensor.matmul(
                    ps_s[:, :nw],
                    _r(qT[:, ki, i * P : (i + 1) * P]),
                    _r(kT[:, ki, ws : ws + nw]),
                    start=(ki == 0),
                    stop=(ki == DC - 1),
                )
            msk_t = p_at.tile([P, 512], BF16, tag="msk", bufs=4)
            nc.sync.dma_start(out=msk_t[:], in_=msk_d[i])
            s_t = p_at.tile([P, 512], F32, tag="s_t", bufs=3)
            nc.vector.tensor_add(out=s_t[:, :nw], in0=ps_s[:, :nw], in1=msk_t[:, :nw])
            attn = p_at.tile([P, 512], BF16, tag="attn", bufs=NT, name=f"attn{i}")
            denom = small.tile([P, 1], F32, tag="denom")
            nc.scalar.activation(
                out=attn[:, :nw], in_=s_t[:, :nw], func=AF.Exp,
                bias=0.0, scale=INV_SQRT_D,
                accum_out=denom[:, 0:1],
            )
            recip = small.tile([P, 1], F32, tag="recip", bufs=NT, name=f"recip{i}")
            nc.vector.reciprocal(out=recip[:], in_=denom[:])
            attns[i] = attn
            recips[i] = recip

        # chunk-granular pipeline: v(j) fills PE while gathers trickle in;
        # scores 0-3 run as soon as t=0 qT/kT exist
        wk_r2 = wk_d[:].rearrange("(k p) o -> p k o", p=P)
        wv_r2 = wv_d[:].rearrange("(k p) o -> p k o", p=P)
        for ki in range(2, DC):
            nc.sync.dma_start(out=wk_sb[:, ki, :], in_=wk_r2[:, ki, :])
        for ki in range(DC):
            nc.sync.dma_start(out=wv_sb[:, ki, :], in_=wv_r2[:, ki, :])
        for k in range(5):
            if k < 4:
                s1_add(k)
            if k >= 1:
                s1_trans(k - 1)
        # adds for chunks 4-7 issued now so DVE never blocks them behind
        # stage-2 eviction work; their transposes run after stage-2 t=0
        for k in range(4, NT):
            s1_add(k)

        # wo load after the latency-critical stage-1 stream
        nc.sync.dma_start(
            out=wo_sb[:], in_=_r(wo_d[:].rearrange("(k p) o -> p k o", p=P))
        )

        s2_qk(wq_sb, bq_sb, qT, 0)
        s2_qk(wk_sb, bk_sb, kT, 0)
        for j in range(4):
            s2_v1(j)
        for i in range(4):
            s3_scores(i)
        for k in range(4, NT):
            s1_trans(k)
            s2_v1(k)
        s2_qk(wq_sb, bq_sb, qT, 1)
        s2_qk(wk_sb, bk_sb, kT, 1)
        for i in range(4, NT):
            s3_scores(i)

        p_wq.release()
        p_qk.release()

        # ----- right-side pools for FFN phase -----
        whpool = tc.alloc_tile_pool(name="whstream", bufs=8, side="right")
        p_h1 = tc.alloc_tile_pool(name="h1p", bufs=1, side="right")
        h1_sb = p_h1.tile([P, NT, D], F32R, tag="h1")
        h1T = p_h1.tile([P, DC, SEQ], BF16, tag="h1T")
        w1_sb = p_h1.tile([P, DC, DFF], BF16, tag="w1")
        nc.sync.dma_start(out=w1_sb[:], in_=w1_d[:].rearrange("(k p) o -> p k o", p=P))
        w2_sb = p_h1.tile([P, FC, D], BF16, tag="w2")
        nc.sync.dma_start(out=w2_sb[:], in_=w2_d[:].rearrange("(k p) o -> p k o", p=P))

        # ---------- stage 3 wave 2 + stage 4, software-pipelined ----------
        p_st4 = tc.alloc_tile_pool(name="st4", bufs=3)
        attnTs = [None] * NT
        o_ts = [None] * NT

        def w2_a(i):  # attn transposes + attnT eviction
            ws = _window_start(i)
            kb0 = ws // P
            nkb = min(DC, i - kb0 + 1)
            pt = psum_t.tile([P, 512], BF16, tag="ptb", bufs=1, name=f"atp{i}")
            for kk in range(nkb):
                nc.tensor.transpose(
                    out=pt[:, kk * P : (kk + 1) * P],
                    in_=attns[i][:, kk * P : (kk + 1) * P],
                    identity=ident_b[:],
                )
            attnT = p_at.tile([P, 512], BF16, tag="attnT", bufs=4, name=f"attnT{i}")
            nc.vector.tensor_copy(out=attnT[:, : nkb * P], in_=pt[:, : nkb * P])
            attnTs[i] = attnT

        def w2_b(i):  # o matmuls + scale
            ws = _window_start(i)
            kb0 = ws // P
            nkb = min(DC, i - kb0 + 1)
            ps_o = psum.tile([P, 512], F32, tag="ps", name=f"pso{i}")
            for kk in range(nkb):
                nc.tensor.matmul(
                    ps_o[:],
                    attnTs[i][:, kk * P : (kk + 1) * P],
                    v_sb[:, kb0 + kk, :],
                    start=(kk == 0),
                    stop=(kk == nkb - 1),
                )
            o_t = p_at.tile([P, D], F32R, tag="o_t", bufs=4, name=f"o_t{i}")
            nc.vector.tensor_scalar_mul(out=o_t[:], in0=ps_o[:], scalar1=recips[i][:, 0:1])
            o_ts[i] = o_t

        def w2_c(i):  # oT transposes + eviction
            pt2 = psum_t.tile([P, 512], F32, tag="pt", name=f"otp{i}")
            for m in range(DC):
                nc.tensor.transpose(
                    out=_r(pt2[:, m * P : (m + 1) * P]),
                    in_=_r(o_ts[i][:, m * P : (m + 1) * P]),
                    identity=_r(ident[:]),
                )
            nc.vector.tensor_copy(out=oT[:, :, i * P : (i + 1) * P], in_=pt2[:])

        def s4_proj(j):  # attn projection + residual + LN1 (no transpose)
            ps = psum.tile([P, 512], F32, tag="ps", name=f"psp{j}")
            for m in range(DC):
                nc.tensor.matmul(
                    ps[:],
                    _r(oT[:, m, j * P : (j + 1) * P]),
                    _r(wo_sb[:, m, :]),
                    start=(m == 0),
                    stop=(m == DC - 1),
                )
            r1 = p_st4.tile([P, D], F32, tag="r1", name=f"r1_{j}")
            nc.vector.tensor_add(out=r1[:], in0=h_sb[:, j, :], in1=ps[:])
            if bo_bc is not None:
                nc.vector.tensor_add(out=r1[:], in0=r1[:], in1=bo_bc[:])
            stats = small.tile([P, 6], F32, tag="stats")
            nc.vector.bn_stats(out=stats[:], in_=r1[:])
            mv = small.tile([P, 2], F32, tag="mv")
            nc.vector.bn_aggr(out=mv[:], in_=stats[:])
            stdt = small.tile([P, 1], F32, tag="stdt")
            nc.scalar.activation(
                out=stdt[:], in_=mv[:, 1:2], func=AF.Sqrt,
                bias=eps_t[:, 0:1], scale=1.0,
            )
            rstd = small.tile([P, 1], F32, tag="rstd")
            nc.vector.reciprocal(out=rstd[:], in_=stdt[:])
            nc.vector.tensor_scalar(
                out=h1_sb[:, j, :], in0=r1[:],
                scalar1=mv[:, 0:1], scalar2=rstd[:, 0:1],
                op0=mybir.AluOpType.subtract, op1=mybir.AluOpType.mult,
            )
            if g1_bc is not None:
                nc.vector.tensor_mul(out=h1_sb[:, j, :], in0=h1_sb[:, j, :], in1=g1_bc[:])
            if be1_bc is not None:
                nc.vector.tensor_add(out=h1_sb[:, j, :], in0=h1_sb[:, j, :], in1=be1_bc[:])

        def s4_trans(j):  # h1 transposes + h1T eviction
            pt3 = psum_t.tile([P, 512], F32, tag="pt", name=f"h1p{j}")
            for m in range(DC):
                nc.tensor.transpose(
                    out=_r(pt3[:, m * P : (m + 1) * P]),
                    in_=_r(h1_sb[:, j, m * P : (m + 1) * P]),
                    identity=_r(ident[:]),
                )
            nc.scalar.copy(out=h1T[:, :, j * P : (j + 1) * P], in_=pt3[:])

        for k in range(NT + 6):
            if k < NT:
                w2_a(k)
            if 1 <= k < NT + 1:
                w2_b(k - 1)
            if 3 <= k < NT + 3:
                w2_c(k - 3)
            if 4 <= k < NT + 4:
                s4_proj(k - 4)
            if 6 <= k:
                s4_trans(k - 6)

        p_st4.release()
        p_at.release()
        p_v.release()
        p_oT.release()
        p_woh.release()

        # ---------- stage 5: FFN up, f1T = relu(W1^T @ h1T + b1) ----------
        p_f1 = tc.alloc_tile_pool(name="f1p", bufs=1, side="right")
        f1T = p_f1.tile([P, FC, SEQ], BF16, tag="f1T")
        def ffn1_group(n, t):
            ps = psum.tile([P, 512], F32, tag="ps", name=f"psf{n}_{t}")
            for ki in range(DC):
                nc.tensor.matmul(
                    ps[:],
                    w1_sb[:, ki, n * P : (n + 1) * P],
                    h1T[:, ki, t * 512 : (t + 1) * 512],
                    start=(ki == 0),
                    stop=(ki == DC - 1),
                )
            fslc = f1T[:, n, t * 512 : (t + 1) * 512]
            if b1_sb is not None:
                nc.vector.tensor_scalar(
                    out=fslc, in0=ps[:],
                    scalar1=b1_sb[:, n : n + 1], scalar2=0.0,
                    op0=mybir.AluOpType.add, op1=mybir.AluOpType.max,
                )
            else:
                nc.vector.tensor_scalar_max(out=fslc, in0=ps[:], scalar1=0.0)

        # ---------- stage 6: FFN down + residual + LN2 (pipelined) ----------
        def s6_main(j):
            ps = psum.tile([P, 512], F32, tag="ps", name=f"ps6_{j}")
            for n in range(FC):
                nc.tensor.matmul(
                    ps[:],
                    f1T[:, n, j * P : (j + 1) * P],
                    w2_sb[:, n, :],
                    start=(n == 0),
                    stop=(n == FC - 1),
                )
            r2 = p_f1.tile([P, D], F32, tag="r2", bufs=3, name=f"r2_{j}")
            nc.vector.tensor_add(out=r2[:], in0=h1_sb[:, j, :], in1=ps[:])
            if b2_bc is not None:
                nc.vector.tensor_add(out=r2[:], in0=r2[:], in1=b2_bc[:])
            stats = small.tile([P, 6], F32, tag="stats")
            nc.vector.bn_stats(out=stats[:], in_=r2[:])
            mv = small.tile([P, 2], F32, tag="mv")
            nc.vector.bn_aggr(out=mv[:], in_=stats[:])
            stdt = small.tile([P, 1], F32, tag="stdt")
            nc.scalar.activation(
                out=stdt[:], in_=mv[:, 1:2], func=AF.Sqrt,
                bias=eps_t[:, 0:1], scale=1.0,
            )
            rstd = small.tile([P, 1], F32, tag="rstd")
            nc.vector.reciprocal(out=rstd[:], in_=stdt[:])
            h2_t = p_f1.tile([P, D], F32R, tag="h2_t", bufs=3, name=f"h2t_{j}")
            nc.vector.tensor_scalar(
                out=h2_t[:], in0=r2[:],
                scalar1=mv[:, 0:1], scalar2=rstd[:, 0:1],
                op0=mybir.AluOpType.subtract, op1=mybir.AluOpType.mult,
            )
            if g2_bc is not None:
                nc.vector.tensor_mul(out=h2_t[:], in0=h2_t[:], in1=g2_bc[:])
            if be2_bc is not None:
                nc.vector.tensor_add(out=h2_t[:], in0=h2_t[:], in1=be2_bc[:])
            return h2_t

        h2ts = [None] * NT

        def s6_trans(j):
            # transpose h2, then split-quantize to fp8: a8T = fp8(h2T),
            # da8T = fp8(h2T - a8T)
            pt = psum_t.tile([P, DC, P], F32, tag="pt", name=f"h2p{j}")
            for m in range(DC):
                nc.tensor.transpose(
                    out=_r(pt[:, m, :]),
                    in_=_r(h2ts[j][:, m * P : (m + 1) * P]),
                    identity=_r(ident[:]),
                )
            nc.scalar.copy(out=a8T[j][:, :, :], in_=pt[:, :, :])
            da_t = p_f1.tile([P, DC, P], F32, tag="da_t", bufs=2, name=f"da_t{j}")
            nc.vector.tensor_sub(out=da_t[:, :, :], in0=pt[:, :, :], in1=a8T[j][:, :, :])
            nc.gpsimd.tensor_copy(out=da8T[j][:, :, :], in_=da_t[:, :, :])

        # head chunks for vc=0,1 interleaved into stage-6 so PE fills LN waits
        def load_whv(vc, nm):
            whv = whpool.tile([P, 2, 2, 2, 256], FP8, tag="whv", name=f"whv{nm}")
            nc.sync.dma_start(out=whv[:], in_=wh8_d[:, vc])
            dwv = whpool.tile([P, 2, 2, 2, 256], FP8, tag="dwv", name=f"dwv{nm}")
            nc.sync.dma_start(out=dwv[:], in_=dwh8_d[:, vc])
            return whv, dwv

        whv0, dwv0 = load_whv(0, "0")
        otile0 = opool.tile([P, NT, 512], BF16 if OUT_BF16 else F32, tag="ot", name="otile0")
        whv1, dwv1 = load_whv(1, "1")
        otile1 = opool.tile([P, NT, 512], BF16 if OUT_BF16 else F32, tag="ot", name="otile1")

        def head_j(whv, dwv, otile, j, toggle):
            # 3-pass fp8 DoubleRow: a@w + a@dw + da@w, one PSUM group per
            # 256-vocab half; scale 1/WH_SCALE folded into the eviction
            ps = psum.tile([P, 512], F32, tag="ps", name=f"psh{toggle}_{j}")
            for t in range(2):
                ops = []
                for ki2 in range(2):
                    lhs_a = a8T[j][:, 2 * ki2 : 2 * ki2 + 2, :]
                    lhs_da = da8T[j][:, 2 * ki2 : 2 * ki2 + 2, :]
                    ops.append((lhs_a, whv[:, t, ki2]))
                    ops.append((lhs_a, dwv[:, t, ki2]))
                    ops.append((lhs_da, whv[:, t, ki2]))
                for n, (l, r) in enumerate(ops):
                    nc.tensor.matmul(
                        ps[:, t * 256 : (t + 1) * 256],
                        l,
                        r,
                        start=(n == 0),
                        stop=(n == len(ops) - 1),
                        perf_mode=DR,
                    )
            if bh_sb_for(toggle) is not None:
                nc.vector.tensor_scalar_mul(
                    out=otile[:, j, :], in0=ps[:], scalar1=1.0 / WH_SCALE
                )
                nc.vector.tensor_add(
                    out=otile[:, j, :], in0=otile[:, j, :], in1=bh_sb_for(toggle)[:]
                )
            elif j % 2 == 0:
                nc.vector.tensor_scalar_mul(
                    out=otile[:, j, :], in0=ps[:], scalar1=1.0 / WH_SCALE
                )
            else:
                nc.scalar.activation(
                    out=otile[:, j, :], in_=ps[:], func=AF.Identity,
                    bias=0.0, scale=1.0 / WH_SCALE,
                )

        _bh_tiles = {}

        def bh_sb_for(key):
            return _bh_tiles.get(key)

        if bh_d is not None:
            bh0 = whpool.tile([P, 512], F32, tag="bh", bufs=2, name="bh0")
            nc.sync.dma_start(out=bh0[:], in_=_bcast_ap(bh_d[0:512]))
            _bh_tiles[0] = bh0
            bh1 = whpool.tile([P, 512], F32, tag="bh", bufs=2, name="bh1")
            nc.sync.dma_start(out=bh1[:], in_=_bcast_ap(bh_d[512:1024]))
            _bh_tiles[1] = bh1

        for t in range(SEQ // 512):
            for n in range(FC):
                ffn1_group(n, t)
                if t == 1 and n % 2 == 1:
                    j = n // 2
                    h2ts[j] = s6_main(j)

        for k in range(NT + 5):
            if 4 <= k < NT:
                h2ts[k] = s6_main(k)
            if 2 <= k <= NT + 1:
                s6_trans(k - 2)
            if 4 <= k <= NT + 3:
                head_j(whv0, dwv0, otile0, k - 4, 0)
            if 5 <= k <= NT + 4:
                head_j(whv1, dwv1, otile1, k - 5, 1)
        out_rr = out_d[:].rearrange("(j p) v -> p j v", p=P)
        nc.sync.dma_start(out=out_rr[:, :, 0:512], in_=otile0[:])
        nc.sync.dma_start(out=out_rr[:, :, 512:1024], in_=otile1[:])

        p_f1.release()
        p_h1.release()

        # ---------- stage 7: vocab head (vc >= 2) ----------
        out_r = out_d[:].rearrange("(j p) v -> p j v", p=P)
        for vc in range(2, NV):
            whv, dwv = load_whv(vc, str(vc))
            if bh_d is not None:
                bh_bc = whpool.tile([P, 512], F32, tag="bh", bufs=2, name=f"bh{vc}")
                nc.sync.dma_start(
                    out=bh_bc[:], in_=_bcast_ap(bh_d[vc * 512 : (vc + 1) * 512])
                )
                _bh_tiles[vc] = bh_bc
            otile = opool.tile([P, NT, 512], BF16 if OUT_BF16 else F32, tag="ot")
            vs = slice(vc * 512, (vc + 1) * 512)
            # split stores per j-half (last chunk: per j-pair) to shrink the
            # final DMA drain after the last matmul
            if vc == NV - 1:
                for j in range(NT):
                    head_j(whv, dwv, otile, j, vc)
                    if j % 2 == 1:
                        nc.sync.dma_start(
                            out=out_r[:, j - 1 : j + 1, vs],
                            in_=otile[:, j - 1 : j + 1, :],
                        )
            else:
                for j in range(NT):
                    head_j(whv, dwv, otile, j, vc)
                    if j == NT // 2 - 1 or j == NT - 1:
                        h0 = j + 1 - NT // 2
                        nc.sync.dma_start(
                            out=out_r[:, h0 : j + 1, vs],
                            in_=otile[:, h0 : j + 1, :],
                        )

        whpool.release()
        p_h2T.release()
        opool.release()
        psum_t.release()
        psum.release()
        small.release()
        const.release()

    nc.finalize()
    return nc


_PROGRAM_CACHE: dict = {}


def _get_program(flags: dict) -> bass.Bass:
    key = tuple(sorted(flags.items()))
    if key not in _PROGRAM_CACHE:
        _PROGRAM_CACHE[key] = _build_program(flags)
    return _PROGRAM_CACHE[key]


def _prep(x, embed_tab, row_embed, col_embed, Wq, bq, Wk, bk, Wv, bv, Wo, bo,
          ln1_g, ln1_b, W1, b1, W2, b2, ln2_g, ln2_b, Wh, bh):
    """Shared host-side prep: flags, common input map, per-core x shards."""
    f32c = lambda a: np.ascontiguousarray(np.asarray(a, dtype=np.float32))
    x = np.asarray(x)
    B = x.shape[0]
    assert x.shape == (B, SEQ)

    import ml_dtypes

    bf16 = ml_dtypes.bfloat16
    fp8 = ml_dtypes.float8_e4m3
    arrs = dict(wo=f32c(Wo))
    arrs["emb"] = np.ascontiguousarray(f32c(embed_tab).astype(bf16))
    arrs["w1"] = np.ascontiguousarray(f32c(W1).astype(bf16))
    arrs["w2"] = np.ascontiguousarray(f32c(W2).astype(bf16))
    arrs["wq"] = np.ascontiguousarray(f32c(Wq).astype(bf16))
    arrs["wk"] = np.ascontiguousarray(f32c(Wk).astype(bf16))
    arrs["wv"] = np.ascontiguousarray(f32c(Wv).astype(bf16))
    # fp8 split head weights, pre-scaled by WH_SCALE:
    #   wh8 = fp8(Wh*S), dwh8 = fp8(Wh*S - wh8); layout [p, vc, t, ki2, i, n]
    whs = f32c(Wh) * WH_SCALE
    w8 = whs.astype(fp8)
    dw8 = (whs - w8.astype(np.float32)).astype(fp8)

    def _wh_layout(a):
        # [D=512, V] -> [ki2, i, p, vc, t, n] -> [p, vc, t, ki2, i, n]
        a = a.reshape(2, 2, P, NV, 2, 256)
        return np.ascontiguousarray(a.transpose(2, 3, 4, 0, 1, 5))

    arrs["wh8"] = _wh_layout(w8)
    arrs["dwh8"] = _wh_layout(dw8)
    pos = np.concatenate(
        [np.repeat(f32c(row_embed), GW, axis=0), np.tile(f32c(col_embed), (GH, 1))],
        axis=-1,
    )
    arrs["pos"] = np.ascontiguousarray(pos.astype(bf16))
    arrs["maskt"] = np.ascontiguousarray(_mask_tiles().astype(bf16))

    bias_map = dict(
        bq=f32c(bq), bk=f32c(bk), bv=f32c(bv), bo=f32c(bo), b1=f32c(b1),
        b2=f32c(b2), bh=f32c(bh), be1=f32c(ln1_b), be2=f32c(ln2_b),
    )
    gain_map = dict(g1=f32c(ln1_g), g2=f32c(ln2_g))
    flags = {k: bool(np.any(v)) for k, v in bias_map.items()}
    flags.update({k: bool(np.any(v != 1.0)) for k, v in gain_map.items()})
    for k, v in {**bias_map, **gain_map}.items():
        if flags[k]:
            arrs[k] = v

    xs = [np.ascontiguousarray(x[c].astype(np.int32)) for c in range(B)]
    return flags, arrs, xs, B


def kernel(**inputs):
    flags, arrs, xs, B = _prep(**inputs)
    nc = _get_program(flags)
    core_ids = list(range(8))
    in_maps = [{**arrs, "x": xs[c % B]} for c in core_ids]
    res = run_bass_kernel_spmd(nc, in_maps, core_ids)
    out = np.stack([res.results[c]["out"] for c in range(B)], axis=0)
    return np.asarray(out, dtype=np.float32)

